# revision 49
# baseline (speedup 1.0000x reference)
"""ModalityUntiedAttention on 8 TRN2 NeuronCores (Bass/Tile).

Sharding: data-parallel over batch (cores 0-3 -> batch 0, cores 4-7 -> batch 1),
tensor-parallel over heads within each 4-core group (4 q heads + 2 kv heads per
core).

Expert (modality) routing: tokens are sorted by modality WITHIN each 512-token
attention group (host-side permutation).  QKV projections run TRANSPOSED
(stationary = weight block, moving = token stream), so each expert streams
exactly its own tokens: expert-0 covers permuted positions [0, e), expert-1
covers [s, 512) where s = min(c0 over the two batches), e = max.  Only the
[s, e) window (|c0_A - c0_B| ~ a dozen tokens) is computed by both experts and
selected with a per-token mask; mixed-tile double compute is eliminated.
The transposed layout is also the attention layout (q^T / k^T with head-dim on
partitions), so no PE transposes are needed for Q/K; RMSNorm uses a
Square + ones-matmul column sum, and RoPE's rotate-half uses an SBUF->SBUF DMA
partition swap.  V is transposed back to natural layout via the PE.

Attention: keys on partitions (scores^T), softmax without max subtraction,
denominator via a ones-column matmul, in-group causal masks via a DVE
multiply.  The wo projection keeps the natural layout (tile kinds pure-0 /
pure-1 / mixed derived from s/e); its partial sums are ReduceScattered (bf16)
over each 4-core group in 512-token chunks (the last group in two 256-token
chunks so the exposed tail RS is halved), RMSNormed on device with deferred
emission so no engine queue blocks on a collective.
"""
import sys

sys.path.insert(0, '/opt/trn_rl_repo')

import os
from contextlib import ExitStack

import numpy as np
import ml_dtypes

import concourse.bass as bass
import concourse.tile as tile
from concourse import bacc, mybir
from concourse.bass import ts, ds, _add_dep_helper
from concourse.bass_utils import run_bass_kernel_spmd
from concourse.masks import make_identity

F32 = mybir.dt.float32
BF16 = mybir.dt.bfloat16

E = 2
HQ = 16
HK = 8
HD = 128
DIM = 2048
BS = 2
SEQ = 2048
EPS = 1e-6

N_CORES = 8
TP = 4                     # cores per batch group
HQC = HQ // TP             # 4 q heads per core
HKC = HK // TP             # 2 kv heads per core
DQ = HQC * HD              # 512 q cols per core
DKV = HKC * HD             # 256 k (and v) cols per core
NT = SEQ // 128            # 16 token tiles
KT = DIM // 128            # 16 contraction tiles
NG = 4                     # 512-token attention groups (= RS chunks)
NCB = HQC + 2 * HKC        # 8 col blocks: 4 q heads, 2 k heads, 2 v heads
GROUPS = [[0, 1, 2, 3], [4, 5, 6, 7]]

_BUILD_CACHE = {}

MUL = mybir.AluOpType.mult
ADD = mybir.AluOpType.add
SUB = mybir.AluOpType.subtract


def _wo_kinds(se):
    """Per-128-token-tile expert kind from the group split points."""
    kinds = []
    for g in range(NG):
        s, e = se[g]
        for t in range(4):
            lo, hi = 128 * t, 128 * (t + 1)
            if hi <= s:
                kinds.append(0)
            elif lo >= e:
                kinds.append(1)
            else:
                kinds.append(2)
    return tuple(kinds)


def build_nc(has_qkw: bool, has_anw: bool, se: tuple):
    """se[g] = (s, e): expert-0 tokens at [0, e), expert-1 at [s, 512) within
    group g's permuted order; [s, e) computed by both and mask-selected."""
    nc = bacc.Bacc("TRN2", target_bir_lowering=False, debug=False,
                   num_devices=N_CORES)

    WSEL = 128
    while any(e - s > WSEL for s, e in se):
        WSEL *= 2

    xg = nc.dram_tensor("xg", [NG, 128, KT, 512], BF16, kind="ExternalInput")
    w0 = nc.dram_tensor("w0", [128, NCB, KT, 128], BF16, kind="ExternalInput")
    w1 = nc.dram_tensor("w1", [128, NCB, KT, 128], BF16, kind="ExternalInput")
    wo0 = nc.dram_tensor("wo0", [DQ, DIM], BF16, kind="ExternalInput")
    wo1 = nc.dram_tensor("wo1", [DQ, DIM], BF16, kind="ExternalInput")
    cosT = nc.dram_tensor("cosT", [128, SEQ], F32, kind="ExternalInput")
    sinT = nc.dram_tensor("sinT", [128, SEQ], F32, kind="ExternalInput")
    mpcw = nc.dram_tensor("mpcw", [128, NG, WSEL], F32, kind="ExternalInput")
    mpc = nc.dram_tensor("mpc", [128, NT], F32, kind="ExternalInput")    # m
    mpc1 = nc.dram_tensor("mpc1", [128, NT], F32, kind="ExternalInput")  # 1-m
    dmin = nc.dram_tensor("dmin", [NT, 128, 512], BF16, kind="ExternalInput")
    if has_qkw:
        # per-token qk norm weights, transposed: [:, 0, t] for q, [:, 1, t] for k
        qkw = nc.dram_tensor("qkw", [128, 2, SEQ], F32, kind="ExternalInput")
    if has_anw:
        anw0 = nc.dram_tensor("anw0", [1, DIM], F32, kind="ExternalInput")
        anwd = nc.dram_tensor("anwd", [1, DIM], F32, kind="ExternalInput")
        mfin = nc.dram_tensor("mfin", [128, NG + 1], F32, kind="ExternalInput")

    out_dram = nc.dram_tensor("out", [SEQ // 4, DIM], F32, kind="ExternalOutput")

    kinds = _wo_kinds(se)

    with tile.TileContext(nc) as tc:
        with ExitStack() as ctx:
            const = ctx.enter_context(tc.tile_pool(name="const", bufs=1))
            persist = ctx.enter_context(tc.tile_pool(name="persist", bufs=1))
            dram = ctx.enter_context(tc.tile_pool(name="dram", bufs=1, space="DRAM"))

            ident = const.tile([128, 128], F32)
            make_identity(nc, ident[:])
            ones_f = const.tile([128, 1], F32)
            nc.gpsimd.memset(ones_f[:], 1.0)
            ones_col = const.tile([128, 1], BF16)
            nc.scalar.copy(ones_col[:], ones_f[:])
            mpc_sb = const.tile([128, NT], F32)
            nc.sync.dma_start(mpc_sb[:], mpc[:, :])
            mpc1_sb = const.tile([128, NT], F32)
            nc.sync.dma_start(mpc1_sb[:], mpc1[:, :])
            eps_q = const.tile([128, 1], F32)
            nc.gpsimd.memset(eps_q[:], float(128.0 * EPS))
            eps_1 = const.tile([128, 1], F32)
            nc.gpsimd.memset(eps_1[:], float(EPS))
            dmasks = const.tile([128, NT, 512], BF16)

            # persistent activation buffers (bf16)
            QT = persist.tile([128, HQC, SEQ], BF16)    # q^T per head (hd, tok)
            KTb = persist.tile([128, HKC, SEQ], BF16)   # k^T per kv head
            Vb = persist.tile([128, NT, DKV], BF16)     # v natural (tok, hd)

            # ------------- Phase 1: QKV projection + norms + rope ------------
            with ExitStack() as p1:
                wpool = p1.enter_context(tc.tile_pool(name="wpool", bufs=1))
                ropec = p1.enter_context(tc.tile_pool(name="ropec", bufs=1))
                xpool = p1.enter_context(tc.tile_pool(name="xpool", bufs=2))
                qkps = p1.enter_context(tc.tile_pool(name="qkps", bufs=2, space="PSUM"))
                csps = p1.enter_context(tc.tile_pool(name="csps", bufs=2, space="PSUM"))
                tps = p1.enter_context(tc.tile_pool(name="tps", bufs=2, space="PSUM"))
                selp = p1.enter_context(tc.tile_pool(name="selp", bufs=3))
                work = p1.enter_context(tc.tile_pool(name="work", bufs=2))

                # mpcw (small, needed by the first evict) leads the gpsimd
                # queue, then weights cb-by-cb; cos/sin ride the sync queue
                # behind the first x chunk so neither delays the first matmul
                mpcw_sb = ropec.tile([128, NG, WSEL], F32)
                nc.gpsimd.dma_start(mpcw_sb[:], mpcw.ap()[:, :, :])
                # tiny dummy collective, triggered at kernel start: absorbs
                # the ~11us first-collective (ncfw) warmup AND the peer launch
                # skew while the collective stream is otherwise idle, so the
                # real (saturated) RS stream later runs at pure data rate
                warm_sb = ropec.tile([4, 128], BF16)
                nc.gpsimd.memset(warm_sb[:], 0.0)
                warm_in = dram.tile([4, 128], BF16, tag="warm_in", bufs=1)
                nc.gpsimd.dma_start(warm_in[:], warm_sb[:])
                warm_out = dram.tile([1, 128], BF16, tag="warm_out", bufs=1)
                nc.gpsimd.collective_compute(
                    "ReduceScatter", mybir.AluOpType.add,
                    replica_groups=GROUPS,
                    ins=[warm_in.opt()], outs=[warm_out.opt()])
                w0_sb = wpool.tile([128, NCB, KT, 128], BF16)
                w1_sb = wpool.tile([128, NCB, KT, 128], BF16)
                # first col block in k-halves so the k=0 matmul starts sooner;
                # late col blocks ride the sync queue (idle after xg/cos/sin)
                # so the unit loop never outruns the weight loads
                nc.gpsimd.dma_start(w0_sb[:, 0, 0:8], w0.ap()[:, 0, 0:8])
                nc.gpsimd.dma_start(w1_sb[:, 0, 0:8], w1.ap()[:, 0, 0:8])
                nc.gpsimd.dma_start(w0_sb[:, 0, 8:KT], w0.ap()[:, 0, 8:KT])
                nc.gpsimd.dma_start(w1_sb[:, 0, 8:KT], w1.ap()[:, 0, 8:KT])
                for cb in range(1, 5):
                    nc.gpsimd.dma_start(w0_sb[:, cb], w0.ap()[:, cb])
                    nc.gpsimd.dma_start(w1_sb[:, cb], w1.ap()[:, cb])
                cos_sb = ropec.tile([128, SEQ], F32)
                sin_sb = ropec.tile([128, SEQ], F32)
                if has_qkw:
                    qkw_sb = ropec.tile([128, 2, SEQ], F32)
                    nc.sync.dma_start(qkw_sb[:], qkw.ap()[:, :, :])

                pend2 = []   # PE work deferred one unit (colsum / transposes)
                pend3 = []   # post-PE chains (sqrt/recip/bcast/rope finish)

                def flush():
                    while pend2:
                        pend2.pop(0)()
                    while pend3:
                        pend3.pop(0)()

                def make_qk_tail(g, cb, t1, ssw, cs_ps):
                    def tail():
                        sqv = work.tile([1, 512], F32, tag="sqv")
                        nc.scalar.activation(
                            sqv[:], cs_ps[:],
                            mybir.ActivationFunctionType.Sqrt,
                            scale=(1.0 if cb < HQC else 1.0 / 128.0),
                            bias=(eps_q if cb < HQC else eps_1)[0:1, :])
                        rs = work.tile([1, 512], F32, tag="rs")
                        nc.vector.reciprocal_approx_fast(rs[:], sqv[:])
                        rs_b = work.tile([128, 512], F32, tag="rs_b")
                        nc.gpsimd.partition_broadcast(rs_b[:], rs[:])
                        t2 = work.tile([128, 512], F32, tag="t2")
                        nc.vector.tensor_mul(t2[:], ssw[:], sin_sb[:, ts(g, 512)])
                        tmp = work.tile([128, 512], F32, tag="tmp")
                        nc.vector.tensor_add(tmp[:], t1[:], t2[:])
                        dst = (QT[:, cb, ts(g, 512)] if cb < HQC
                               else KTb[:, cb - HQC, ts(g, 512)])
                        nc.vector.tensor_mul(dst, tmp[:], rs_b[:])
                    return tail

                for g in range(NG):
                    s, e = se[g]
                    wA, wB = e, 512 - s
                    xg_sb = xpool.tile([128, KT, 512], BF16, tag="xg")
                    if g == 0:
                        nc.sync.dma_start(xg_sb[:, 0:4], xg.ap()[g, :, 0:4])
                        nc.sync.dma_start(xg_sb[:, 4:KT], xg.ap()[g, :, 4:KT])
                        nc.sync.dma_start(cos_sb[:], cosT[:, :])
                        nc.sync.dma_start(sin_sb[:], sinT[:, :])
                        for cb in range(5, NCB):
                            nc.sync.dma_start(w0_sb[:, cb], w0.ap()[:, cb])
                            nc.sync.dma_start(w1_sb[:, cb], w1.ap()[:, cb])
                    else:
                        nc.sync.dma_start(xg_sb[:], xg.ap()[g])

                    for cb in range(NCB):
                        psA = psB = None
                        if wA:
                            psA = qkps.tile([128, 512], F32, tag="psA")
                        if wB:
                            psB = qkps.tile([128, 512], F32, tag="psB")
                        for k in range(KT):
                            st, sp = k == 0, k == KT - 1
                            if wA:
                                nc.tensor.matmul(
                                    psA[:, 0:wA], w0_sb[:, cb, k, :],
                                    xg_sb[:, k, 0:wA], start=st, stop=sp)
                            if wB:
                                nc.tensor.matmul(
                                    psB[:, 0:wB], w1_sb[:, cb, k, :],
                                    xg_sb[:, k, s:512], start=st, stop=sp)
                        # deferred PE + chains of the previous unit run here,
                        # covered by this unit's matmuls
                        flush()

                        # evict with expert select on the [s, e) window
                        sel = selp.tile([128, 512], F32, tag="sel")
                        if s > 0:
                            nc.vector.tensor_copy(sel[:, 0:s], psA[:, 0:s])
                        if e < 512:
                            nc.vector.tensor_copy(sel[:, e:512], psB[:, e - s:512 - s])
                        if e > s:
                            # DVE may read only one PSUM operand per op
                            wtB = work.tile([128, WSEL], F32, tag="wtB")
                            nc.vector.tensor_copy(wtB[:, 0:e - s], psB[:, 0:e - s])
                            wt = work.tile([128, WSEL], F32, tag="wt")
                            nc.vector.tensor_sub(
                                wt[:, 0:e - s], wtB[:, 0:e - s], psA[:, s:e])
                            nc.vector.tensor_mul(
                                wt[:, 0:e - s], wt[:, 0:e - s],
                                mpcw_sb[:, g, 0:e - s])
                            nc.vector.tensor_add(
                                sel[:, s:e], psA[:, s:e], wt[:, 0:e - s])

                        if cb < HQC + HKC:
                            # q/k: rms stats + rope now; finish deferred
                            sq = selp.tile([128, 512], BF16, tag="sq")
                            nc.scalar.activation(
                                sq[:], sel[:],
                                mybir.ActivationFunctionType.Square)
                            if has_qkw:
                                # norm weight applies before rope (rope mixes
                                # hd pairs); rms stats are pre-weight
                                nc.vector.tensor_mul(
                                    sel[:], sel[:],
                                    qkw_sb[:, 0 if cb < HQC else 1, ts(g, 512)])
                            ssw = selp.tile([128, 512], F32, tag="ssw")
                            nc.sync.dma_start(ssw[0:64, :], sel[64:128, :])
                            nc.sync.dma_start(ssw[64:128, :], sel[0:64, :])
                            t1 = work.tile([128, 512], F32, tag="t1")
                            nc.vector.tensor_mul(t1[:], sel[:], cos_sb[:, ts(g, 512)])

                            cs_ps = csps.tile([1, 512], F32, tag="cs")

                            def colsum(cs_ps=cs_ps, sq=sq):
                                nc.tensor.matmul(cs_ps[:], ones_col[:], sq[:],
                                                 start=True, stop=True)
                            pend2.append(colsum)
                            pend3.append(make_qk_tail(g, cb, t1, ssw, cs_ps))
                        else:
                            # v: transpose back to natural layout
                            vcb = cb - HQC - HKC
                            tp = tps.tile([128, 4, 128], F32, tag="tp")

                            def vtrans(tp=tp, sel=sel):
                                for j in range(4):
                                    nc.tensor.transpose(
                                        tp[:, j, :], sel[:, ts(j, 128)], ident[:])
                            pend2.append(vtrans)

                            def vcopy(tp=tp, g=g, vcb=vcb):
                                nc.vector.tensor_copy(
                                    Vb[:, ds(4 * g, 4), ts(vcb, 128)], tp[:])
                            pend3.append(vcopy)
                flush()

            # ------------- Phase 2+3: attention + wo + RS + final norm -------
            with ExitStack() as p23:
                wopool = p23.enter_context(tc.tile_pool(name="wopool", bufs=1))
                ofp = p23.enter_context(tc.tile_pool(name="ofp", bufs=1))
                sps = p23.enter_context(tc.tile_pool(name="sps", bufs=2, space="PSUM"))
                otps = p23.enter_context(tc.tile_pool(name="otps", bufs=1, space="PSUM"))
                dnps = p23.enter_context(tc.tile_pool(name="dnps", bufs=1, space="PSUM"))
                wops = p23.enter_context(tc.tile_pool(name="wops", bufs=2, space="PSUM"))
                probs = p23.enter_context(tc.tile_pool(name="probs", bufs=8))
                redc = p23.enter_context(tc.tile_pool(name="redc", bufs=3))
                att = p23.enter_context(tc.tile_pool(name="att", bufs=2))
                opool = p23.enter_context(tc.tile_pool(name="opool", bufs=3))
                npool = p23.enter_context(tc.tile_pool(name="npool", bufs=2))

                ofT = ofp.tile([128, HQC, SEQ], BF16)   # out_flat^T (hd, tok)
                nc.gpsimd.dma_start(dmasks[:], dmin.ap().rearrange("t p f -> p t f"))

                wo0_sb = wopool.tile([128, 4, DIM], BF16)
                nc.sync.dma_start(wo0_sb[:], wo0.ap().rearrange("(k p) f -> p k f", p=128))
                wo1_sb = wopool.tile([128, 4, DIM], BF16)
                nc.sync.dma_start(wo1_sb[:], wo1.ap().rearrange("(k p) f -> p k f", p=128))
                if has_anw:
                    anw0_sb = wopool.tile([1, DIM], F32)
                    nc.sync.dma_start(anw0_sb[:], anw0[:, :])
                    anwd_sb = wopool.tile([1, DIM], F32)
                    nc.sync.dma_start(anwd_sb[:], anwd[:, :])
                    anw0_b = wopool.tile([128, DIM], F32)
                    nc.gpsimd.partition_broadcast(anw0_b[:], anw0_sb[:])
                    anwd_b = wopool.tile([128, DIM], F32)
                    nc.gpsimd.partition_broadcast(anwd_b[:], anwd_sb[:])
                    mfin_sb = wopool.tile([128, NG + 1], F32)
                    nc.sync.dma_start(mfin_sb[:], mfin[:, :])

                pending_rs = []
                pending_den = []

                def do_den(g, h, dn_ps):
                    den = att.tile([1, 512], F32, tag="den")
                    nc.vector.reciprocal_approx_fast(den[:], dn_ps[:])
                    den_b = att.tile([128, 512], F32, tag="den_b")
                    nc.gpsimd.partition_broadcast(den_b[:], den[:])
                    nc.vector.tensor_mul(
                        ofT[:, h, ts(g, 512)], ofT[:, h, ts(g, 512)], den_b[:])

                def do_final_norm(row0, nrows, mf, rs_out, dep=None):
                    sum_sb = npool.tile([nrows, DIM], BF16, tag=f"sum_sb{nrows}")
                    first = nc.sync.dma_start(sum_sb[:], rs_out[:])
                    if dep is not None:
                        _add_dep_helper(first.ins, dep.ins, sync=False,
                                        reason="norms run only after all wo chunks")
                    fin = npool.tile([nrows, DIM], F32, tag=f"fin{nrows}")
                    z = npool.tile([nrows, 1], F32, tag=f"z{nrows}")
                    nc.vector.scalar_tensor_tensor(
                        out=fin[:], in0=sum_sb[:], scalar=1.0, in1=sum_sb[:],
                        op0=MUL, op1=MUL, accum_out=z[:])
                    sz = npool.tile([nrows, 1], F32, tag=f"sz{nrows}")
                    nc.scalar.activation(sz[:], z[:],
                                         mybir.ActivationFunctionType.Sqrt,
                                         scale=1.0 / float(DIM),
                                         bias=eps_1[0:nrows, :])
                    rz = npool.tile([nrows, 1], F32, tag=f"rz{nrows}")
                    nc.vector.reciprocal_approx_fast(rz[:], sz[:])
                    nc.scalar.mul(fin[:], sum_sb[:], rz[:])
                    if has_anw:
                        anw_sel = npool.tile([nrows, DIM], F32, tag="anw_sel")
                        nc.vector.scalar_tensor_tensor(
                            out=anw_sel[:], in0=anwd_b[0:nrows, :],
                            scalar=mfin_sb[0:nrows, mf:mf + 1],
                            in1=anw0_b[0:nrows, :], op0=MUL, op1=ADD)
                        nc.vector.tensor_mul(fin[:], fin[:], anw_sel[:])
                    nc.sync.dma_start(out_dram.ap()[ds(row0, nrows), :], fin[:])

                for g in range(NG):
                    for h in range(HQC):
                        kv = h // (HQC // HKC)
                        njt = 4 * (g + 1)
                        ot_ps = otps.tile([128, 512], F32, tag="ot")
                        dn_ps = dnps.tile([1, 512], F32, tag="dn")
                        dn_hold = []
                        dn_first = True
                        for jp in range(njt // 2):
                            j0 = 2 * jp
                            # two 512-score tiles in one 2-bank psum tile so
                            # exp and the causal-mask multiply run 1024 wide
                            s_ps = sps.tile([128, 2, 512], F32, tag="s")
                            for dj in range(2):
                                nc.tensor.matmul(
                                    s_ps[:, dj, :], KTb[:, kv, ts(j0 + dj, 128)],
                                    QT[:, h, ts(g, 512)], start=True, stop=True)
                            p_t = probs.tile([128, 2, 512], BF16, tag="p")
                            nc.scalar.activation(
                                p_t[:], s_ps[:], mybir.ActivationFunctionType.Exp)
                            if j0 >= 4 * g:
                                pm_t = probs.tile([128, 2, 512], BF16, tag="pm")
                                nc.vector.tensor_mul(
                                    pm_t[:], p_t[:],
                                    dmasks[:, ds(j0, 2), :])
                                p_t = pm_t
                            # denominator: DVE-pre-reduce up to 8 key tiles,
                            # then one 512-row matmul per batch
                            ps_pair = redc.tile([128, 512], BF16, tag="pp",
                                                bufs=4)
                            nc.vector.tensor_add(
                                ps_pair[:], p_t[:, 0, :], p_t[:, 1, :])
                            dn_hold.append(ps_pair)
                            if len(dn_hold) == 4 or jp == njt // 2 - 1:
                                while len(dn_hold) > 1:
                                    a = dn_hold.pop(0)
                                    b = dn_hold.pop(0)
                                    ps4 = redc.tile([128, 512], BF16, tag="p4")
                                    nc.vector.tensor_add(ps4[:], a[:], b[:])
                                    dn_hold.append(ps4)
                                nc.tensor.matmul(
                                    dn_ps[:], ones_col[:], dn_hold.pop()[:],
                                    start=dn_first,
                                    stop=(jp == njt // 2 - 1))
                                dn_first = False
                            for dj in range(2):
                                j = j0 + dj
                                st = j == 0
                                sp = j == njt - 1
                                nc.tensor.matmul(
                                    ot_ps[:], Vb[:, j, ts(kv, 128)], p_t[:, dj, :],
                                    start=st, stop=sp)
                        # fast raw evict frees the psum; normalization deferred
                        nc.vector.tensor_copy(ofT[:, h, ts(g, 512)], ot_ps[:])
                        pending_den.append((h, dn_ps))
                        if len(pending_den) > 1:
                            do_den(g, *pending_den.pop(0))

                    while pending_den:
                        do_den(g, *pending_den.pop(0))

                    # wo projection for this chunk (two small tail RS proved
                    # slower than one 2MB RS: ~10us fixed cost per collective)
                    nch = 1
                    for ch in range(nch):
                        nt_ch = 4 // nch
                        rs_in = dram.tile([128 * nt_ch, DIM], BF16,
                                          tag=f"rs_in{nt_ch}", bufs=4)
                        for u in range(nt_ch):
                            T = 4 * g + nt_ch * ch + u
                            kind = kinds[T]
                            woa_sb = wo1_sb if kind == 1 else wo0_sb
                            o_sb = opool.tile([128, DIM], BF16, tag="o_sb")
                            for n in range(4):
                                wo_ps = wops.tile([128, 512], F32, tag="wop")
                                if kind == 2:
                                    wb_ps = wops.tile([128, 512], F32, tag="wop")
                                for kk in range(4):
                                    nc.tensor.matmul(
                                        wo_ps[:], ofT[:, kk, ts(T, 128)],
                                        woa_sb[:, kk, ts(n, 512)],
                                        start=(kk == 0), stop=(kk == 3))
                                    if kind == 2:
                                        nc.tensor.matmul(
                                            wb_ps[:], ofT[:, kk, ts(T, 128)],
                                            wo1_sb[:, kk, ts(n, 512)],
                                            start=(kk == 0), stop=(kk == 3))
                                if kind == 2:
                                    nc.scalar.mul(o_sb[:, ts(n, 512)], wo_ps[:],
                                                  mpc1_sb[:, T:T + 1])
                                    nc.vector.scalar_tensor_tensor(
                                        out=o_sb[:, ts(n, 512)], in0=wb_ps[:],
                                        scalar=mpc_sb[:, T:T + 1],
                                        in1=o_sb[:, ts(n, 512)], op0=MUL, op1=ADD)
                                else:
                                    nc.scalar.copy(o_sb[:, ts(n, 512)], wo_ps[:])
                            last_rsin_dma = nc.sync.dma_start(
                                rs_in[ts(u, 128), :], o_sb[:])

                        rs_out = dram.tile([32 * nt_ch, DIM], BF16,
                                           tag=f"rs_out{nt_ch}", bufs=4)
                        nc.gpsimd.collective_compute(
                            "ReduceScatter", mybir.AluOpType.add,
                            replica_groups=GROUPS,
                            ins=[rs_in.opt()], outs=[rs_out.opt()])
                        mf = g if g < NG - 1 else NG - 1 + ch
                        pending_rs.append(
                            (128 * g + 32 * nt_ch * ch, 32 * nt_ch, mf, rs_out))

                # ALL final norms run after the last wo: nothing consumes a
                # ReduceScatter result until every chunk is in flight, so the
                # in-order engine queues never block on a collective even when
                # peer cores launch with large skew (the first RS would
                # otherwise wait for laggards and stall attention mid-phase).
                # The Tile scheduler reorders by priority, so emission order
                # alone is not enough: push priority to the maximum (appear
                # last) and chain the first DMA after the final wo output.
                with tc.high_priority(offset=-(1 << 20)):
                    for args in pending_rs:
                        do_final_norm(*args, dep=last_rsin_dma)

    nc.compile()
    return nc


def _plan(modality_ids):
    """Per-group stable modality sort; shared split points across batches."""
    mids = np.asarray(modality_ids).reshape(BS, SEQ)
    perms = np.empty((BS, SEQ), np.int64)   # permuted pos -> original token idx
    c0 = np.empty((BS, NG), np.int64)
    for b in range(BS):
        for G in range(NG):
            mg = mids[b, 512 * G:512 * (G + 1)]
            i0 = np.where(mg == 0)[0]
            i1 = np.where(mg == 1)[0]
            c0[b, G] = len(i0)
            perms[b, 512 * G:512 * (G + 1)] = 512 * G + np.concatenate([i0, i1])
    se = tuple((int(c0[:, G].min()), int(c0[:, G].max())) for G in range(NG))
    return perms, se


def _prep_inputs(x, freqs_cos, freqs_sin, wq, wk, wv, wo,
                 q_norm_w, k_norm_w, attn_norm_w, modality_ids,
                 has_qkw, has_anw, perms, se):
    """Build the 8 per-core input maps (numpy marshaling only)."""
    x = np.asarray(x, np.float32)
    freqs_cos = np.asarray(freqs_cos, np.float32)
    freqs_sin = np.asarray(freqs_sin, np.float32)
    wq = np.asarray(wq, np.float32)
    wk = np.asarray(wk, np.float32)
    wv = np.asarray(wv, np.float32)
    wo = np.asarray(wo, np.float32)
    mids = np.asarray(modality_ids).reshape(BS, SEQ)

    WSEL = 128
    while any(e - s > WSEL for s, e in se):
        WSEL *= 2

    # de-interleave the hd dimension: [even dims, odd dims]
    perm_hd = np.concatenate([np.arange(0, HD, 2), np.arange(1, HD, 2)])

    def permute_heads(w, nh):
        w4 = w.reshape(E, DIM, nh, HD)
        return w4[:, :, :, perm_hd].reshape(E, DIM, nh * HD)

    wq_p = permute_heads(wq, HQ)
    wk_p = permute_heads(wk, HK)
    wv_p = permute_heads(wv, HK)
    wo4 = wo.reshape(E, HQ, HD, DIM)[:, :, perm_hd, :].reshape(E, HQ * HD, DIM)

    cosT_full = np.concatenate([freqs_cos.T, freqs_cos.T], axis=0)   # (HD, SEQ)
    sinT_full = np.concatenate([-freqs_sin.T, freqs_sin.T], axis=0)

    in_maps = []
    for c in range(N_CORES):
        b, r = divmod(c, TP)
        P = perms[b]
        qs = slice(r * DQ, (r + 1) * DQ)
        ks = slice(r * DKV, (r + 1) * DKV)
        w0c = np.concatenate(
            [wq_p[0][:, qs], wk_p[0][:, ks], wv_p[0][:, ks]], axis=1)
        w1c = np.concatenate(
            [wq_p[1][:, qs], wk_p[1][:, ks], wv_p[1][:, ks]], axis=1)
        m = mids[b].astype(np.float32)[P]
        # in-group causal masks for the permuted order
        pos = (P % 512)
        dmv = np.zeros((NT, 128, 512), np.float32)
        for j in range(NT):
            gj = j // 4
            kpos = pos[128 * j:128 * (j + 1)]
            qpos = pos[512 * gj:512 * (gj + 1)]
            dmv[j] = (kpos[:, None] <= qpos[None, :])
        # expert-select window masks (broadcast down partitions)
        mw = np.zeros((128, NG, WSEL), np.float32)
        for G in range(NG):
            s, e = se[G]
            mw[:, G, 0:e - s] = m[512 * G + s:512 * G + e][None, :]
        xp = x[b].T[:, P]   # (DIM, SEQ) permuted tokens
        im = {
            "xg": np.ascontiguousarray(
                xp.reshape(KT, 128, NG, 512).transpose(2, 1, 0, 3)
            ).astype(ml_dtypes.bfloat16),
            "w0": np.ascontiguousarray(
                w0c.reshape(KT, 128, NCB, 128).transpose(1, 2, 0, 3)
            ).astype(ml_dtypes.bfloat16),
            "w1": np.ascontiguousarray(
                w1c.reshape(KT, 128, NCB, 128).transpose(1, 2, 0, 3)
            ).astype(ml_dtypes.bfloat16),
            "wo0": wo4[0][r * DQ:(r + 1) * DQ, :].astype(ml_dtypes.bfloat16),
            "wo1": wo4[1][r * DQ:(r + 1) * DQ, :].astype(ml_dtypes.bfloat16),
            "cosT": np.ascontiguousarray(cosT_full[:, P]),
            "sinT": np.ascontiguousarray(sinT_full[:, P]),
            "mpcw": mw,
            "mpc": np.ascontiguousarray(m.reshape(NT, 128).T),
            "mpc1": np.ascontiguousarray((1.0 - m).reshape(NT, 128).T),
            "dmin": dmv.astype(ml_dtypes.bfloat16),
        }
        if has_qkw:
            qw = np.asarray(q_norm_w, np.float32)[:, perm_hd]
            kw = np.asarray(k_norm_w, np.float32)[:, perm_hd]
            msel = mids[b][P]
            qkwv = np.empty((128, 2, SEQ), np.float32)
            qkwv[:, 0, :] = qw[msel].T
            qkwv[:, 1, :] = kw[msel].T
            im["qkw"] = qkwv
        if has_anw:
            aw = np.asarray(attn_norm_w, np.float32)
            im["anw0"] = np.ascontiguousarray(aw[0:1])
            im["anwd"] = (aw[1] - aw[0]).reshape(1, DIM).copy()
            mf = np.zeros((128, NG + 1), np.float32)
            for g in range(NG):
                t0 = 512 * g + 128 * r
                mf[:, g] = m[t0:t0 + 128]
            im["mfin"] = mf
        in_maps.append(im)
    return in_maps


def kernel(**inputs):
    q_norm_w = np.asarray(inputs["q_norm_w"], np.float32)
    k_norm_w = np.asarray(inputs["k_norm_w"], np.float32)
    attn_norm_w = np.asarray(inputs["attn_norm_w"], np.float32)
    has_qkw = not (np.all(q_norm_w == 1.0) and np.all(k_norm_w == 1.0))
    has_anw = not np.all(attn_norm_w == 1.0)

    perms, se = _plan(inputs["modality_ids"])
    key = (has_qkw, has_anw, se)
    if key not in _BUILD_CACHE:
        _BUILD_CACHE[key] = build_nc(has_qkw, has_anw, se)
    nc = _BUILD_CACHE[key]

    in_maps = _prep_inputs(
        inputs["x"], inputs["freqs_cos"], inputs["freqs_sin"],
        inputs["wq"], inputs["wk"], inputs["wv"], inputs["wo"],
        q_norm_w, k_norm_w, attn_norm_w, inputs["modality_ids"],
        has_qkw, has_anw, perms, se)

    res = run_bass_kernel_spmd(nc, in_maps, core_ids=list(range(N_CORES)))

    out = np.empty((BS, SEQ, DIM), np.float32)
    for c in range(N_CORES):
        b, r = divmod(c, TP)
        P = perms[b]
        oc = res.results[c]["out"]          # (SEQ//4, DIM), permuted rows
        for g in range(NG):
            t0 = 512 * g + 128 * r          # permuted-space positions
            out[b, P[t0:t0 + 128], :] = oc[128 * g:128 * (g + 1), :]
    return out


# revision 51
# speedup vs baseline: 1.0062x; 1.0062x over previous
"""ModalityUntiedAttention on 8 TRN2 NeuronCores (Bass/Tile).

Sharding: data-parallel over batch (cores 0-3 -> batch 0, cores 4-7 -> batch 1),
tensor-parallel over heads within each 4-core group (4 q heads + 2 kv heads per
core).

Expert (modality) routing: tokens are sorted by modality WITHIN each 512-token
attention group (host-side permutation).  QKV projections run TRANSPOSED
(stationary = weight block, moving = token stream), so each expert streams
exactly its own tokens: expert-0 covers permuted positions [0, e), expert-1
covers [s, 512) where s = min(c0 over the two batches), e = max.  Only the
[s, e) window (|c0_A - c0_B| ~ a dozen tokens) is computed by both experts and
selected with a per-token mask; mixed-tile double compute is eliminated.
The transposed layout is also the attention layout (q^T / k^T with head-dim on
partitions), so no PE transposes are needed for Q/K; RMSNorm uses a
Square + ones-matmul column sum, and RoPE's rotate-half uses an SBUF->SBUF DMA
partition swap.  V is transposed back to natural layout via the PE.

Attention: keys on partitions (scores^T), softmax without max subtraction,
denominator via a ones-column matmul, in-group causal masks via a DVE
multiply.  The wo projection keeps the natural layout (tile kinds pure-0 /
pure-1 / mixed derived from s/e); its partial sums are ReduceScattered (bf16)
over each 4-core group in 512-token chunks (the last group in two 256-token
chunks so the exposed tail RS is halved), RMSNormed on device with deferred
emission so no engine queue blocks on a collective.
"""
import sys

sys.path.insert(0, '/opt/trn_rl_repo')

import os
from contextlib import ExitStack

import numpy as np
import ml_dtypes

import concourse.bass as bass
import concourse.tile as tile
from concourse import bacc, mybir
from concourse.bass import ts, ds, _add_dep_helper
from concourse.bass_utils import run_bass_kernel_spmd
from concourse.masks import make_identity

F32 = mybir.dt.float32
BF16 = mybir.dt.bfloat16

E = 2
HQ = 16
HK = 8
HD = 128
DIM = 2048
BS = 2
SEQ = 2048
EPS = 1e-6

N_CORES = 8
TP = 4                     # cores per batch group
HQC = HQ // TP             # 4 q heads per core
HKC = HK // TP             # 2 kv heads per core
DQ = HQC * HD              # 512 q cols per core
DKV = HKC * HD             # 256 k (and v) cols per core
NT = SEQ // 128            # 16 token tiles
KT = DIM // 128            # 16 contraction tiles
NG = 4                     # 512-token attention groups (= RS chunks)
NCB = HQC + 2 * HKC        # 8 col blocks: 4 q heads, 2 k heads, 2 v heads
GROUPS = [[0, 1, 2, 3], [4, 5, 6, 7]]

_BUILD_CACHE = {}

MUL = mybir.AluOpType.mult
ADD = mybir.AluOpType.add
SUB = mybir.AluOpType.subtract


def _wo_kinds(se):
    """Per-128-token-tile expert kind from the group split points."""
    kinds = []
    for g in range(NG):
        s, e = se[g]
        for t in range(4):
            lo, hi = 128 * t, 128 * (t + 1)
            if hi <= s:
                kinds.append(0)
            elif lo >= e:
                kinds.append(1)
            else:
                kinds.append(2)
    return tuple(kinds)


def build_nc(has_qkw: bool, has_anw: bool, se: tuple):
    """se[g] = (s, e): expert-0 tokens at [0, e), expert-1 at [s, 512) within
    group g's permuted order; [s, e) computed by both and mask-selected."""
    nc = bacc.Bacc("TRN2", target_bir_lowering=False, debug=False,
                   num_devices=N_CORES)

    WSEL = 128
    while any(e - s > WSEL for s, e in se):
        WSEL *= 2

    xg = nc.dram_tensor("xg", [NG, 128, KT, 512], BF16, kind="ExternalInput")
    w0 = nc.dram_tensor("w0", [128, NCB, KT, 128], BF16, kind="ExternalInput")
    w1 = nc.dram_tensor("w1", [128, NCB, KT, 128], BF16, kind="ExternalInput")
    wo0 = nc.dram_tensor("wo0", [DQ, DIM], BF16, kind="ExternalInput")
    wo1 = nc.dram_tensor("wo1", [DQ, DIM], BF16, kind="ExternalInput")
    cosT = nc.dram_tensor("cosT", [128, SEQ], F32, kind="ExternalInput")
    sinT = nc.dram_tensor("sinT", [128, SEQ], F32, kind="ExternalInput")
    mpcw = nc.dram_tensor("mpcw", [128, NG, WSEL], F32, kind="ExternalInput")
    mpc = nc.dram_tensor("mpc", [128, NT], F32, kind="ExternalInput")    # m
    mpc1 = nc.dram_tensor("mpc1", [128, NT], F32, kind="ExternalInput")  # 1-m
    dmin = nc.dram_tensor("dmin", [NT, 128, 512], BF16, kind="ExternalInput")
    if has_qkw:
        # per-token qk norm weights, transposed: [:, 0, t] for q, [:, 1, t] for k
        qkw = nc.dram_tensor("qkw", [128, 2, SEQ], F32, kind="ExternalInput")
    if has_anw:
        anw0 = nc.dram_tensor("anw0", [1, DIM], F32, kind="ExternalInput")
        anwd = nc.dram_tensor("anwd", [1, DIM], F32, kind="ExternalInput")
        mfin = nc.dram_tensor("mfin", [128, NG + 1], F32, kind="ExternalInput")

    out_dram = nc.dram_tensor("out", [SEQ // 4, DIM], F32, kind="ExternalOutput")

    kinds = _wo_kinds(se)

    with tile.TileContext(nc) as tc:
        with ExitStack() as ctx:
            const = ctx.enter_context(tc.tile_pool(name="const", bufs=1))
            persist = ctx.enter_context(tc.tile_pool(name="persist", bufs=1))
            dram = ctx.enter_context(tc.tile_pool(name="dram", bufs=1, space="DRAM"))

            ident = const.tile([128, 128], F32)
            make_identity(nc, ident[:])
            ones_f = const.tile([128, 1], F32)
            nc.gpsimd.memset(ones_f[:], 1.0)
            ones_col = const.tile([128, 1], BF16)
            nc.scalar.copy(ones_col[:], ones_f[:])
            mpc_sb = const.tile([128, NT], F32)
            nc.sync.dma_start(mpc_sb[:], mpc[:, :])
            mpc1_sb = const.tile([128, NT], F32)
            nc.sync.dma_start(mpc1_sb[:], mpc1[:, :])
            eps_q = const.tile([128, 1], F32)
            nc.gpsimd.memset(eps_q[:], float(128.0 * EPS))
            eps_1 = const.tile([128, 1], F32)
            nc.gpsimd.memset(eps_1[:], float(EPS))
            dmasks = const.tile([128, NT, 512], BF16)

            # persistent activation buffers (bf16)
            QT = persist.tile([128, HQC, SEQ], BF16)    # q^T per head (hd, tok)
            KTb = persist.tile([128, HKC, SEQ], BF16)   # k^T per kv head
            Vb = persist.tile([128, NT, DKV], BF16)     # v natural (tok, hd)

            # ------------- Phase 1: QKV projection + norms + rope ------------
            with ExitStack() as p1:
                wpool = p1.enter_context(tc.tile_pool(name="wpool", bufs=1))
                ropec = p1.enter_context(tc.tile_pool(name="ropec", bufs=1))
                xpool = p1.enter_context(tc.tile_pool(name="xpool", bufs=2))
                qkps = p1.enter_context(tc.tile_pool(name="qkps", bufs=2, space="PSUM"))
                csps = p1.enter_context(tc.tile_pool(name="csps", bufs=2, space="PSUM"))
                tps = p1.enter_context(tc.tile_pool(name="tps", bufs=2, space="PSUM"))
                selp = p1.enter_context(tc.tile_pool(name="selp", bufs=3))
                work = p1.enter_context(tc.tile_pool(name="work", bufs=2))

                # mpcw (small, needed by the first evict) leads the gpsimd
                # queue, then weights cb-by-cb; cos/sin ride the sync queue
                # behind the first x chunk so neither delays the first matmul
                mpcw_sb = ropec.tile([128, NG, WSEL], F32)
                nc.gpsimd.dma_start(mpcw_sb[:], mpcw.ap()[:, :, :])
                w0_sb = wpool.tile([128, NCB, KT, 128], BF16)
                w1_sb = wpool.tile([128, NCB, KT, 128], BF16)
                # first col block in k-halves so the k=0 matmul starts sooner;
                # late col blocks ride the sync queue (idle after xg/cos/sin)
                # so the unit loop never outruns the weight loads
                nc.gpsimd.dma_start(w0_sb[:, 0, 0:8], w0.ap()[:, 0, 0:8])
                nc.gpsimd.dma_start(w1_sb[:, 0, 0:8], w1.ap()[:, 0, 0:8])
                nc.gpsimd.dma_start(w0_sb[:, 0, 8:KT], w0.ap()[:, 0, 8:KT])
                nc.gpsimd.dma_start(w1_sb[:, 0, 8:KT], w1.ap()[:, 0, 8:KT])
                for cb in range(1, 5):
                    nc.gpsimd.dma_start(w0_sb[:, cb], w0.ap()[:, cb])
                    nc.gpsimd.dma_start(w1_sb[:, cb], w1.ap()[:, cb])
                # tiny dummy collective AFTER all phase-1 gpsimd DMAs (its
                # trigger sync blocks this queue until peers arrive): absorbs
                # the ~11us first-collective warmup and the peer launch skew
                # while the gpsimd queue and collective stream are idle, so
                # the real (saturated) RS stream later runs at pure data rate
                warm_sb = ropec.tile([4, 128], BF16)
                nc.gpsimd.memset(warm_sb[:], 0.0)
                warm_in = dram.tile([4, 128], BF16, tag="warm_in", bufs=1)
                nc.gpsimd.dma_start(warm_in[:], warm_sb[:])
                warm_out = dram.tile([1, 128], BF16, tag="warm_out", bufs=1)
                nc.gpsimd.collective_compute(
                    "ReduceScatter", mybir.AluOpType.add,
                    replica_groups=GROUPS,
                    ins=[warm_in.opt()], outs=[warm_out.opt()])
                cos_sb = ropec.tile([128, SEQ], F32)
                sin_sb = ropec.tile([128, SEQ], F32)
                if has_qkw:
                    qkw_sb = ropec.tile([128, 2, SEQ], F32)
                    nc.sync.dma_start(qkw_sb[:], qkw.ap()[:, :, :])

                pend2 = []   # PE work deferred one unit (colsum / transposes)
                pend3 = []   # post-PE chains (sqrt/recip/bcast/rope finish)

                def flush():
                    while pend2:
                        pend2.pop(0)()
                    while pend3:
                        pend3.pop(0)()

                def make_qk_tail(g, cb, t1, ssw, cs_ps):
                    def tail():
                        sqv = work.tile([1, 512], F32, tag="sqv")
                        nc.scalar.activation(
                            sqv[:], cs_ps[:],
                            mybir.ActivationFunctionType.Sqrt,
                            scale=(1.0 if cb < HQC else 1.0 / 128.0),
                            bias=(eps_q if cb < HQC else eps_1)[0:1, :])
                        rs = work.tile([1, 512], F32, tag="rs")
                        nc.vector.reciprocal_approx_fast(rs[:], sqv[:])
                        rs_b = work.tile([128, 512], F32, tag="rs_b")
                        nc.gpsimd.partition_broadcast(rs_b[:], rs[:])
                        t2 = work.tile([128, 512], F32, tag="t2")
                        nc.vector.tensor_mul(t2[:], ssw[:], sin_sb[:, ts(g, 512)])
                        tmp = work.tile([128, 512], F32, tag="tmp")
                        nc.vector.tensor_add(tmp[:], t1[:], t2[:])
                        dst = (QT[:, cb, ts(g, 512)] if cb < HQC
                               else KTb[:, cb - HQC, ts(g, 512)])
                        nc.vector.tensor_mul(dst, tmp[:], rs_b[:])
                    return tail

                for g in range(NG):
                    s, e = se[g]
                    wA, wB = e, 512 - s
                    xg_sb = xpool.tile([128, KT, 512], BF16, tag="xg")
                    if g == 0:
                        nc.sync.dma_start(xg_sb[:, 0:4], xg.ap()[g, :, 0:4])
                        nc.sync.dma_start(xg_sb[:, 4:KT], xg.ap()[g, :, 4:KT])
                        nc.sync.dma_start(cos_sb[:], cosT[:, :])
                        nc.sync.dma_start(sin_sb[:], sinT[:, :])
                        for cb in range(5, NCB):
                            nc.sync.dma_start(w0_sb[:, cb], w0.ap()[:, cb])
                            nc.sync.dma_start(w1_sb[:, cb], w1.ap()[:, cb])
                    else:
                        nc.sync.dma_start(xg_sb[:], xg.ap()[g])

                    for cb in range(NCB):
                        psA = psB = None
                        if wA:
                            psA = qkps.tile([128, 512], F32, tag="psA")
                        if wB:
                            psB = qkps.tile([128, 512], F32, tag="psB")
                        for k in range(KT):
                            st, sp = k == 0, k == KT - 1
                            if wA:
                                nc.tensor.matmul(
                                    psA[:, 0:wA], w0_sb[:, cb, k, :],
                                    xg_sb[:, k, 0:wA], start=st, stop=sp)
                            if wB:
                                nc.tensor.matmul(
                                    psB[:, 0:wB], w1_sb[:, cb, k, :],
                                    xg_sb[:, k, s:512], start=st, stop=sp)
                        # deferred PE + chains of the previous unit run here,
                        # covered by this unit's matmuls
                        flush()

                        # evict with expert select on the [s, e) window
                        sel = selp.tile([128, 512], F32, tag="sel")
                        if s > 0:
                            nc.vector.tensor_copy(sel[:, 0:s], psA[:, 0:s])
                        if e < 512:
                            nc.vector.tensor_copy(sel[:, e:512], psB[:, e - s:512 - s])
                        if e > s:
                            # DVE may read only one PSUM operand per op
                            wtB = work.tile([128, WSEL], F32, tag="wtB")
                            nc.vector.tensor_copy(wtB[:, 0:e - s], psB[:, 0:e - s])
                            wt = work.tile([128, WSEL], F32, tag="wt")
                            nc.vector.tensor_sub(
                                wt[:, 0:e - s], wtB[:, 0:e - s], psA[:, s:e])
                            nc.vector.tensor_mul(
                                wt[:, 0:e - s], wt[:, 0:e - s],
                                mpcw_sb[:, g, 0:e - s])
                            nc.vector.tensor_add(
                                sel[:, s:e], psA[:, s:e], wt[:, 0:e - s])

                        if cb < HQC + HKC:
                            # q/k: rms stats + rope now; finish deferred
                            sq = selp.tile([128, 512], BF16, tag="sq")
                            nc.scalar.activation(
                                sq[:], sel[:],
                                mybir.ActivationFunctionType.Square)
                            if has_qkw:
                                # norm weight applies before rope (rope mixes
                                # hd pairs); rms stats are pre-weight
                                nc.vector.tensor_mul(
                                    sel[:], sel[:],
                                    qkw_sb[:, 0 if cb < HQC else 1, ts(g, 512)])
                            ssw = selp.tile([128, 512], F32, tag="ssw")
                            nc.sync.dma_start(ssw[0:64, :], sel[64:128, :])
                            nc.sync.dma_start(ssw[64:128, :], sel[0:64, :])
                            t1 = work.tile([128, 512], F32, tag="t1")
                            nc.vector.tensor_mul(t1[:], sel[:], cos_sb[:, ts(g, 512)])

                            cs_ps = csps.tile([1, 512], F32, tag="cs")

                            def colsum(cs_ps=cs_ps, sq=sq):
                                nc.tensor.matmul(cs_ps[:], ones_col[:], sq[:],
                                                 start=True, stop=True)
                            pend2.append(colsum)
                            pend3.append(make_qk_tail(g, cb, t1, ssw, cs_ps))
                        else:
                            # v: transpose back to natural layout
                            vcb = cb - HQC - HKC
                            tp = tps.tile([128, 4, 128], F32, tag="tp")

                            def vtrans(tp=tp, sel=sel):
                                for j in range(4):
                                    nc.tensor.transpose(
                                        tp[:, j, :], sel[:, ts(j, 128)], ident[:])
                            pend2.append(vtrans)

                            def vcopy(tp=tp, g=g, vcb=vcb):
                                nc.vector.tensor_copy(
                                    Vb[:, ds(4 * g, 4), ts(vcb, 128)], tp[:])
                            pend3.append(vcopy)
                flush()

            # ------------- Phase 2+3: attention + wo + RS + final norm -------
            with ExitStack() as p23:
                wopool = p23.enter_context(tc.tile_pool(name="wopool", bufs=1))
                ofp = p23.enter_context(tc.tile_pool(name="ofp", bufs=1))
                sps = p23.enter_context(tc.tile_pool(name="sps", bufs=2, space="PSUM"))
                otps = p23.enter_context(tc.tile_pool(name="otps", bufs=1, space="PSUM"))
                dnps = p23.enter_context(tc.tile_pool(name="dnps", bufs=1, space="PSUM"))
                wops = p23.enter_context(tc.tile_pool(name="wops", bufs=2, space="PSUM"))
                probs = p23.enter_context(tc.tile_pool(name="probs", bufs=8))
                redc = p23.enter_context(tc.tile_pool(name="redc", bufs=3))
                att = p23.enter_context(tc.tile_pool(name="att", bufs=2))
                opool = p23.enter_context(tc.tile_pool(name="opool", bufs=3))
                npool = p23.enter_context(tc.tile_pool(name="npool", bufs=2))

                ofT = ofp.tile([128, HQC, SEQ], BF16)   # out_flat^T (hd, tok)
                nc.gpsimd.dma_start(dmasks[:], dmin.ap().rearrange("t p f -> p t f"))

                wo0_sb = wopool.tile([128, 4, DIM], BF16)
                nc.sync.dma_start(wo0_sb[:], wo0.ap().rearrange("(k p) f -> p k f", p=128))
                wo1_sb = wopool.tile([128, 4, DIM], BF16)
                nc.sync.dma_start(wo1_sb[:], wo1.ap().rearrange("(k p) f -> p k f", p=128))
                if has_anw:
                    anw0_sb = wopool.tile([1, DIM], F32)
                    nc.sync.dma_start(anw0_sb[:], anw0[:, :])
                    anwd_sb = wopool.tile([1, DIM], F32)
                    nc.sync.dma_start(anwd_sb[:], anwd[:, :])
                    anw0_b = wopool.tile([128, DIM], F32)
                    nc.gpsimd.partition_broadcast(anw0_b[:], anw0_sb[:])
                    anwd_b = wopool.tile([128, DIM], F32)
                    nc.gpsimd.partition_broadcast(anwd_b[:], anwd_sb[:])
                    mfin_sb = wopool.tile([128, NG + 1], F32)
                    nc.sync.dma_start(mfin_sb[:], mfin[:, :])

                pending_rs = []
                pending_den = []

                def do_den(g, h, dn_ps):
                    den = att.tile([1, 512], F32, tag="den")
                    nc.vector.reciprocal_approx_fast(den[:], dn_ps[:])
                    den_b = att.tile([128, 512], F32, tag="den_b")
                    nc.gpsimd.partition_broadcast(den_b[:], den[:])
                    nc.vector.tensor_mul(
                        ofT[:, h, ts(g, 512)], ofT[:, h, ts(g, 512)], den_b[:])

                def do_final_norm(row0, nrows, mf, rs_out, dep=None):
                    sum_sb = npool.tile([nrows, DIM], BF16, tag=f"sum_sb{nrows}")
                    first = nc.sync.dma_start(sum_sb[:], rs_out[:])
                    if dep is not None:
                        _add_dep_helper(first.ins, dep.ins, sync=False,
                                        reason="norms run only after all wo chunks")
                    fin = npool.tile([nrows, DIM], F32, tag=f"fin{nrows}")
                    z = npool.tile([nrows, 1], F32, tag=f"z{nrows}")
                    nc.vector.scalar_tensor_tensor(
                        out=fin[:], in0=sum_sb[:], scalar=1.0, in1=sum_sb[:],
                        op0=MUL, op1=MUL, accum_out=z[:])
                    sz = npool.tile([nrows, 1], F32, tag=f"sz{nrows}")
                    nc.scalar.activation(sz[:], z[:],
                                         mybir.ActivationFunctionType.Sqrt,
                                         scale=1.0 / float(DIM),
                                         bias=eps_1[0:nrows, :])
                    rz = npool.tile([nrows, 1], F32, tag=f"rz{nrows}")
                    nc.vector.reciprocal_approx_fast(rz[:], sz[:])
                    nc.scalar.mul(fin[:], sum_sb[:], rz[:])
                    if has_anw:
                        anw_sel = npool.tile([nrows, DIM], F32, tag="anw_sel")
                        nc.vector.scalar_tensor_tensor(
                            out=anw_sel[:], in0=anwd_b[0:nrows, :],
                            scalar=mfin_sb[0:nrows, mf:mf + 1],
                            in1=anw0_b[0:nrows, :], op0=MUL, op1=ADD)
                        nc.vector.tensor_mul(fin[:], fin[:], anw_sel[:])
                    nc.sync.dma_start(out_dram.ap()[ds(row0, nrows), :], fin[:])

                for g in range(NG):
                    for h in range(HQC):
                        kv = h // (HQC // HKC)
                        njt = 4 * (g + 1)
                        ot_ps = otps.tile([128, 512], F32, tag="ot")
                        dn_ps = dnps.tile([1, 512], F32, tag="dn")
                        dn_hold = []
                        dn_first = True
                        for jp in range(njt // 2):
                            j0 = 2 * jp
                            # two 512-score tiles in one 2-bank psum tile so
                            # exp and the causal-mask multiply run 1024 wide
                            s_ps = sps.tile([128, 2, 512], F32, tag="s")
                            for dj in range(2):
                                nc.tensor.matmul(
                                    s_ps[:, dj, :], KTb[:, kv, ts(j0 + dj, 128)],
                                    QT[:, h, ts(g, 512)], start=True, stop=True)
                            p_t = probs.tile([128, 2, 512], BF16, tag="p")
                            nc.scalar.activation(
                                p_t[:], s_ps[:], mybir.ActivationFunctionType.Exp)
                            if j0 >= 4 * g:
                                pm_t = probs.tile([128, 2, 512], BF16, tag="pm")
                                nc.vector.tensor_mul(
                                    pm_t[:], p_t[:],
                                    dmasks[:, ds(j0, 2), :])
                                p_t = pm_t
                            # denominator: DVE-pre-reduce up to 8 key tiles,
                            # then one 512-row matmul per batch
                            ps_pair = redc.tile([128, 512], BF16, tag="pp",
                                                bufs=4)
                            nc.vector.tensor_add(
                                ps_pair[:], p_t[:, 0, :], p_t[:, 1, :])
                            dn_hold.append(ps_pair)
                            if len(dn_hold) == 4 or jp == njt // 2 - 1:
                                while len(dn_hold) > 1:
                                    a = dn_hold.pop(0)
                                    b = dn_hold.pop(0)
                                    ps4 = redc.tile([128, 512], BF16, tag="p4")
                                    nc.vector.tensor_add(ps4[:], a[:], b[:])
                                    dn_hold.append(ps4)
                                nc.tensor.matmul(
                                    dn_ps[:], ones_col[:], dn_hold.pop()[:],
                                    start=dn_first,
                                    stop=(jp == njt // 2 - 1))
                                dn_first = False
                            for dj in range(2):
                                j = j0 + dj
                                st = j == 0
                                sp = j == njt - 1
                                nc.tensor.matmul(
                                    ot_ps[:], Vb[:, j, ts(kv, 128)], p_t[:, dj, :],
                                    start=st, stop=sp)
                        # fast raw evict frees the psum; normalization deferred
                        nc.vector.tensor_copy(ofT[:, h, ts(g, 512)], ot_ps[:])
                        pending_den.append((h, dn_ps))
                        if len(pending_den) > 1:
                            do_den(g, *pending_den.pop(0))

                    while pending_den:
                        do_den(g, *pending_den.pop(0))

                    # wo projection for this chunk (two small tail RS proved
                    # slower than one 2MB RS: ~10us fixed cost per collective)
                    nch = 1
                    for ch in range(nch):
                        nt_ch = 4 // nch
                        rs_in = dram.tile([128 * nt_ch, DIM], BF16,
                                          tag=f"rs_in{nt_ch}", bufs=4)
                        for u in range(nt_ch):
                            T = 4 * g + nt_ch * ch + u
                            kind = kinds[T]
                            woa_sb = wo1_sb if kind == 1 else wo0_sb
                            o_sb = opool.tile([128, DIM], BF16, tag="o_sb")
                            for n in range(4):
                                wo_ps = wops.tile([128, 512], F32, tag="wop")
                                if kind == 2:
                                    wb_ps = wops.tile([128, 512], F32, tag="wop")
                                for kk in range(4):
                                    nc.tensor.matmul(
                                        wo_ps[:], ofT[:, kk, ts(T, 128)],
                                        woa_sb[:, kk, ts(n, 512)],
                                        start=(kk == 0), stop=(kk == 3))
                                    if kind == 2:
                                        nc.tensor.matmul(
                                            wb_ps[:], ofT[:, kk, ts(T, 128)],
                                            wo1_sb[:, kk, ts(n, 512)],
                                            start=(kk == 0), stop=(kk == 3))
                                if kind == 2:
                                    nc.scalar.mul(o_sb[:, ts(n, 512)], wo_ps[:],
                                                  mpc1_sb[:, T:T + 1])
                                    nc.vector.scalar_tensor_tensor(
                                        out=o_sb[:, ts(n, 512)], in0=wb_ps[:],
                                        scalar=mpc_sb[:, T:T + 1],
                                        in1=o_sb[:, ts(n, 512)], op0=MUL, op1=ADD)
                                else:
                                    nc.scalar.copy(o_sb[:, ts(n, 512)], wo_ps[:])
                            last_rsin_dma = nc.sync.dma_start(
                                rs_in[ts(u, 128), :], o_sb[:])

                        rs_out = dram.tile([32 * nt_ch, DIM], BF16,
                                           tag=f"rs_out{nt_ch}", bufs=4)
                        nc.gpsimd.collective_compute(
                            "ReduceScatter", mybir.AluOpType.add,
                            replica_groups=GROUPS,
                            ins=[rs_in.opt()], outs=[rs_out.opt()])
                        mf = g if g < NG - 1 else NG - 1 + ch
                        pending_rs.append(
                            (128 * g + 32 * nt_ch * ch, 32 * nt_ch, mf, rs_out))

                # ALL final norms run after the last wo: nothing consumes a
                # ReduceScatter result until every chunk is in flight, so the
                # in-order engine queues never block on a collective even when
                # peer cores launch with large skew (the first RS would
                # otherwise wait for laggards and stall attention mid-phase).
                # The Tile scheduler reorders by priority, so emission order
                # alone is not enough: push priority to the maximum (appear
                # last) and chain the first DMA after the final wo output.
                with tc.high_priority(offset=-(1 << 20)):
                    for args in pending_rs:
                        do_final_norm(*args, dep=last_rsin_dma)

    nc.compile()
    return nc


def _plan(modality_ids):
    """Per-group stable modality sort; shared split points across batches."""
    mids = np.asarray(modality_ids).reshape(BS, SEQ)
    perms = np.empty((BS, SEQ), np.int64)   # permuted pos -> original token idx
    c0 = np.empty((BS, NG), np.int64)
    for b in range(BS):
        for G in range(NG):
            mg = mids[b, 512 * G:512 * (G + 1)]
            i0 = np.where(mg == 0)[0]
            i1 = np.where(mg == 1)[0]
            c0[b, G] = len(i0)
            perms[b, 512 * G:512 * (G + 1)] = 512 * G + np.concatenate([i0, i1])
    se = tuple((int(c0[:, G].min()), int(c0[:, G].max())) for G in range(NG))
    return perms, se


def _prep_inputs(x, freqs_cos, freqs_sin, wq, wk, wv, wo,
                 q_norm_w, k_norm_w, attn_norm_w, modality_ids,
                 has_qkw, has_anw, perms, se):
    """Build the 8 per-core input maps (numpy marshaling only)."""
    x = np.asarray(x, np.float32)
    freqs_cos = np.asarray(freqs_cos, np.float32)
    freqs_sin = np.asarray(freqs_sin, np.float32)
    wq = np.asarray(wq, np.float32)
    wk = np.asarray(wk, np.float32)
    wv = np.asarray(wv, np.float32)
    wo = np.asarray(wo, np.float32)
    mids = np.asarray(modality_ids).reshape(BS, SEQ)

    WSEL = 128
    while any(e - s > WSEL for s, e in se):
        WSEL *= 2

    # de-interleave the hd dimension: [even dims, odd dims]
    perm_hd = np.concatenate([np.arange(0, HD, 2), np.arange(1, HD, 2)])

    def permute_heads(w, nh):
        w4 = w.reshape(E, DIM, nh, HD)
        return w4[:, :, :, perm_hd].reshape(E, DIM, nh * HD)

    wq_p = permute_heads(wq, HQ)
    wk_p = permute_heads(wk, HK)
    wv_p = permute_heads(wv, HK)
    wo4 = wo.reshape(E, HQ, HD, DIM)[:, :, perm_hd, :].reshape(E, HQ * HD, DIM)

    cosT_full = np.concatenate([freqs_cos.T, freqs_cos.T], axis=0)   # (HD, SEQ)
    sinT_full = np.concatenate([-freqs_sin.T, freqs_sin.T], axis=0)

    in_maps = []
    for c in range(N_CORES):
        b, r = divmod(c, TP)
        P = perms[b]
        qs = slice(r * DQ, (r + 1) * DQ)
        ks = slice(r * DKV, (r + 1) * DKV)
        w0c = np.concatenate(
            [wq_p[0][:, qs], wk_p[0][:, ks], wv_p[0][:, ks]], axis=1)
        w1c = np.concatenate(
            [wq_p[1][:, qs], wk_p[1][:, ks], wv_p[1][:, ks]], axis=1)
        m = mids[b].astype(np.float32)[P]
        # in-group causal masks for the permuted order
        pos = (P % 512)
        dmv = np.zeros((NT, 128, 512), np.float32)
        for j in range(NT):
            gj = j // 4
            kpos = pos[128 * j:128 * (j + 1)]
            qpos = pos[512 * gj:512 * (gj + 1)]
            dmv[j] = (kpos[:, None] <= qpos[None, :])
        # expert-select window masks (broadcast down partitions)
        mw = np.zeros((128, NG, WSEL), np.float32)
        for G in range(NG):
            s, e = se[G]
            mw[:, G, 0:e - s] = m[512 * G + s:512 * G + e][None, :]
        xp = x[b].T[:, P]   # (DIM, SEQ) permuted tokens
        im = {
            "xg": np.ascontiguousarray(
                xp.reshape(KT, 128, NG, 512).transpose(2, 1, 0, 3)
            ).astype(ml_dtypes.bfloat16),
            "w0": np.ascontiguousarray(
                w0c.reshape(KT, 128, NCB, 128).transpose(1, 2, 0, 3)
            ).astype(ml_dtypes.bfloat16),
            "w1": np.ascontiguousarray(
                w1c.reshape(KT, 128, NCB, 128).transpose(1, 2, 0, 3)
            ).astype(ml_dtypes.bfloat16),
            "wo0": wo4[0][r * DQ:(r + 1) * DQ, :].astype(ml_dtypes.bfloat16),
            "wo1": wo4[1][r * DQ:(r + 1) * DQ, :].astype(ml_dtypes.bfloat16),
            "cosT": np.ascontiguousarray(cosT_full[:, P]),
            "sinT": np.ascontiguousarray(sinT_full[:, P]),
            "mpcw": mw,
            "mpc": np.ascontiguousarray(m.reshape(NT, 128).T),
            "mpc1": np.ascontiguousarray((1.0 - m).reshape(NT, 128).T),
            "dmin": dmv.astype(ml_dtypes.bfloat16),
        }
        if has_qkw:
            qw = np.asarray(q_norm_w, np.float32)[:, perm_hd]
            kw = np.asarray(k_norm_w, np.float32)[:, perm_hd]
            msel = mids[b][P]
            qkwv = np.empty((128, 2, SEQ), np.float32)
            qkwv[:, 0, :] = qw[msel].T
            qkwv[:, 1, :] = kw[msel].T
            im["qkw"] = qkwv
        if has_anw:
            aw = np.asarray(attn_norm_w, np.float32)
            im["anw0"] = np.ascontiguousarray(aw[0:1])
            im["anwd"] = (aw[1] - aw[0]).reshape(1, DIM).copy()
            mf = np.zeros((128, NG + 1), np.float32)
            for g in range(NG):
                t0 = 512 * g + 128 * r
                mf[:, g] = m[t0:t0 + 128]
            im["mfin"] = mf
        in_maps.append(im)
    return in_maps


def kernel(**inputs):
    q_norm_w = np.asarray(inputs["q_norm_w"], np.float32)
    k_norm_w = np.asarray(inputs["k_norm_w"], np.float32)
    attn_norm_w = np.asarray(inputs["attn_norm_w"], np.float32)
    has_qkw = not (np.all(q_norm_w == 1.0) and np.all(k_norm_w == 1.0))
    has_anw = not np.all(attn_norm_w == 1.0)

    perms, se = _plan(inputs["modality_ids"])
    key = (has_qkw, has_anw, se)
    if key not in _BUILD_CACHE:
        _BUILD_CACHE[key] = build_nc(has_qkw, has_anw, se)
    nc = _BUILD_CACHE[key]

    in_maps = _prep_inputs(
        inputs["x"], inputs["freqs_cos"], inputs["freqs_sin"],
        inputs["wq"], inputs["wk"], inputs["wv"], inputs["wo"],
        q_norm_w, k_norm_w, attn_norm_w, inputs["modality_ids"],
        has_qkw, has_anw, perms, se)

    res = run_bass_kernel_spmd(nc, in_maps, core_ids=list(range(N_CORES)))

    out = np.empty((BS, SEQ, DIM), np.float32)
    for c in range(N_CORES):
        b, r = divmod(c, TP)
        P = perms[b]
        oc = res.results[c]["out"]          # (SEQ//4, DIM), permuted rows
        for g in range(NG):
            t0 = 512 * g + 128 * r          # permuted-space positions
            out[b, P[t0:t0 + 128], :] = oc[128 * g:128 * (g + 1), :]
    return out


# revision 52
# speedup vs baseline: 1.0523x; 1.0458x over previous
"""ModalityUntiedAttention on 8 TRN2 NeuronCores (Bass/Tile).

Sharding: data-parallel over batch (cores 0-3 -> batch 0, cores 4-7 -> batch 1),
tensor-parallel over heads within each 4-core group (4 q heads + 2 kv heads per
core).

Expert (modality) routing: tokens are sorted by modality WITHIN each 512-token
attention group (host-side permutation).  QKV projections run TRANSPOSED
(stationary = weight block, moving = token stream), so each expert streams
exactly its own tokens: expert-0 covers permuted positions [0, e), expert-1
covers [s, 512) where s = min(c0 over the two batches), e = max.  Only the
[s, e) window (|c0_A - c0_B| ~ a dozen tokens) is computed by both experts and
selected with a per-token mask; mixed-tile double compute is eliminated.
The transposed layout is also the attention layout (q^T / k^T with head-dim on
partitions), so no PE transposes are needed for Q/K; RMSNorm uses a
Square + ones-matmul column sum, and RoPE's rotate-half uses an SBUF->SBUF DMA
partition swap.  V is transposed back to natural layout via the PE.

Attention: keys on partitions (scores^T), softmax without max subtraction,
denominator via a ones-column matmul, in-group causal masks via a DVE
multiply.  The wo projection keeps the natural layout (tile kinds pure-0 /
pure-1 / mixed derived from s/e); its partial sums are ReduceScattered (bf16)
over each 4-core group in 512-token chunks (the last group in two 256-token
chunks so the exposed tail RS is halved), RMSNormed on device with deferred
emission so no engine queue blocks on a collective.
"""
import sys

sys.path.insert(0, '/opt/trn_rl_repo')

import os
from contextlib import ExitStack

import numpy as np
import ml_dtypes

import concourse.bass as bass
import concourse.tile as tile
from concourse import bacc, mybir
from concourse.bass import ts, ds, _add_dep_helper
from concourse.bass_utils import run_bass_kernel_spmd
from concourse.masks import make_identity

F32 = mybir.dt.float32
BF16 = mybir.dt.bfloat16

E = 2
HQ = 16
HK = 8
HD = 128
DIM = 2048
BS = 2
SEQ = 2048
EPS = 1e-6

N_CORES = 8
TP = 4                     # cores per batch group
HQC = HQ // TP             # 4 q heads per core
HKC = HK // TP             # 2 kv heads per core
DQ = HQC * HD              # 512 q cols per core
DKV = HKC * HD             # 256 k (and v) cols per core
NT = SEQ // 128            # 16 token tiles
KT = DIM // 128            # 16 contraction tiles
NG = 4                     # 512-token attention groups (= RS chunks)
NCB = HQC + 2 * HKC        # 8 col blocks: 4 q heads, 2 k heads, 2 v heads
GROUPS = [[0, 1, 2, 3], [4, 5, 6, 7]]

_BUILD_CACHE = {}

MUL = mybir.AluOpType.mult
ADD = mybir.AluOpType.add
SUB = mybir.AluOpType.subtract


def _wo_kinds(se):
    """Per-128-token-tile expert kind from the group split points."""
    kinds = []
    for g in range(NG):
        s, e = se[g]
        for t in range(4):
            lo, hi = 128 * t, 128 * (t + 1)
            if hi <= s:
                kinds.append(0)
            elif lo >= e:
                kinds.append(1)
            else:
                kinds.append(2)
    return tuple(kinds)


def build_nc(has_qkw: bool, has_anw: bool, se: tuple):
    """se[g] = (s, e): expert-0 tokens at [0, e), expert-1 at [s, 512) within
    group g's permuted order; [s, e) computed by both and mask-selected."""
    nc = bacc.Bacc("TRN2", target_bir_lowering=False, debug=False,
                   num_devices=N_CORES)

    WSEL = 128
    while any(e - s > WSEL for s, e in se):
        WSEL *= 2

    xg = nc.dram_tensor("xg", [NG, 128, KT, 512], BF16, kind="ExternalInput")
    w0 = nc.dram_tensor("w0", [128, NCB, KT, 128], BF16, kind="ExternalInput")
    w1 = nc.dram_tensor("w1", [128, NCB, KT, 128], BF16, kind="ExternalInput")
    wo0 = nc.dram_tensor("wo0", [DQ, DIM], BF16, kind="ExternalInput")
    wo1 = nc.dram_tensor("wo1", [DQ, DIM], BF16, kind="ExternalInput")
    cosT = nc.dram_tensor("cosT", [128, SEQ], F32, kind="ExternalInput")
    sinT = nc.dram_tensor("sinT", [128, SEQ], F32, kind="ExternalInput")
    mpcw = nc.dram_tensor("mpcw", [128, NG, WSEL], F32, kind="ExternalInput")
    mpc = nc.dram_tensor("mpc", [128, NT], F32, kind="ExternalInput")    # m
    mpc1 = nc.dram_tensor("mpc1", [128, NT], F32, kind="ExternalInput")  # 1-m
    dmin = nc.dram_tensor("dmin", [NT, 128, 512], BF16, kind="ExternalInput")
    if has_qkw:
        # per-token qk norm weights, transposed: [:, 0, t] for q, [:, 1, t] for k
        qkw = nc.dram_tensor("qkw", [128, 2, SEQ], F32, kind="ExternalInput")
    if has_anw:
        anw0 = nc.dram_tensor("anw0", [1, DIM], F32, kind="ExternalInput")
        anwd = nc.dram_tensor("anwd", [1, DIM], F32, kind="ExternalInput")
        mfin = nc.dram_tensor("mfin", [128, NG + 1], F32, kind="ExternalInput")

    out_dram = nc.dram_tensor("out", [SEQ // 4, DIM], F32, kind="ExternalOutput")

    kinds = _wo_kinds(se)

    with tile.TileContext(nc) as tc:
        with ExitStack() as ctx:
            const = ctx.enter_context(tc.tile_pool(name="const", bufs=1))
            persist = ctx.enter_context(tc.tile_pool(name="persist", bufs=1))
            dram = ctx.enter_context(tc.tile_pool(name="dram", bufs=1, space="DRAM"))

            ident = const.tile([128, 128], F32)
            make_identity(nc, ident[:])
            ones_f = const.tile([128, 1], F32)
            nc.gpsimd.memset(ones_f[:], 1.0)
            ones_col = const.tile([128, 1], BF16)
            nc.scalar.copy(ones_col[:], ones_f[:])
            mpc_sb = const.tile([128, NT], F32)
            nc.sync.dma_start(mpc_sb[:], mpc[:, :])
            mpc1_sb = const.tile([128, NT], F32)
            nc.sync.dma_start(mpc1_sb[:], mpc1[:, :])
            eps_q = const.tile([128, 1], F32)
            nc.gpsimd.memset(eps_q[:], float(128.0 * EPS))
            eps_1 = const.tile([128, 1], F32)
            nc.gpsimd.memset(eps_1[:], float(EPS))
            dmasks = const.tile([128, NT, 512], BF16)

            # persistent activation buffers (bf16)
            QT = persist.tile([128, HQC, SEQ], BF16)    # q^T per head (hd, tok)
            KTb = persist.tile([128, HKC, SEQ], BF16)   # k^T per kv head
            Vb = persist.tile([128, NT, DKV], BF16)     # v natural (tok, hd)

            # ------------- Phase 1: QKV projection + norms + rope ------------
            with ExitStack() as p1:
                wpool = p1.enter_context(tc.tile_pool(name="wpool", bufs=1))
                ropec = p1.enter_context(tc.tile_pool(name="ropec", bufs=1))
                xpool = p1.enter_context(tc.tile_pool(name="xpool", bufs=2))
                qkps = p1.enter_context(tc.tile_pool(name="qkps", bufs=2, space="PSUM"))
                csps = p1.enter_context(tc.tile_pool(name="csps", bufs=2, space="PSUM"))
                tps = p1.enter_context(tc.tile_pool(name="tps", bufs=2, space="PSUM"))
                selp = p1.enter_context(tc.tile_pool(name="selp", bufs=3))
                work = p1.enter_context(tc.tile_pool(name="work", bufs=2))

                # mpcw (small, needed by the first evict) leads the gpsimd
                # queue, then weights cb-by-cb; cos/sin ride the sync queue
                # behind the first x chunk so neither delays the first matmul
                mpcw_sb = ropec.tile([128, NG, WSEL], F32)
                nc.gpsimd.dma_start(mpcw_sb[:], mpcw.ap()[:, :, :])
                w0_sb = wpool.tile([128, NCB, KT, 128], BF16)
                w1_sb = wpool.tile([128, NCB, KT, 128], BF16)
                # first col block in k-halves so the k=0 matmul starts sooner;
                # late col blocks ride the sync queue (idle after xg/cos/sin)
                # so the unit loop never outruns the weight loads
                nc.gpsimd.dma_start(w0_sb[:, 0, 0:8], w0.ap()[:, 0, 0:8])
                nc.gpsimd.dma_start(w1_sb[:, 0, 0:8], w1.ap()[:, 0, 0:8])
                nc.gpsimd.dma_start(w0_sb[:, 0, 8:KT], w0.ap()[:, 0, 8:KT])
                nc.gpsimd.dma_start(w1_sb[:, 0, 8:KT], w1.ap()[:, 0, 8:KT])
                for cb in range(1, 5):
                    nc.gpsimd.dma_start(w0_sb[:, cb], w0.ap()[:, cb])
                    nc.gpsimd.dma_start(w1_sb[:, cb], w1.ap()[:, cb])
                cos_sb = ropec.tile([128, SEQ], F32)
                sin_sb = ropec.tile([128, SEQ], F32)
                if has_qkw:
                    qkw_sb = ropec.tile([128, 2, SEQ], F32)
                    nc.sync.dma_start(qkw_sb[:], qkw.ap()[:, :, :])

                pend2 = []   # PE work deferred one unit (colsum / transposes)
                pend3 = []   # post-PE chains (sqrt/recip/bcast/rope finish)

                def flush():
                    while pend2:
                        pend2.pop(0)()
                    while pend3:
                        pend3.pop(0)()

                def make_qk_tail(g, cb, t1, ssw, cs_ps):
                    def tail():
                        sqv = work.tile([1, 512], F32, tag="sqv")
                        nc.scalar.activation(
                            sqv[:], cs_ps[:],
                            mybir.ActivationFunctionType.Sqrt,
                            scale=(1.0 if cb < HQC else 1.0 / 128.0),
                            bias=(eps_q if cb < HQC else eps_1)[0:1, :])
                        rs = work.tile([1, 512], F32, tag="rs")
                        nc.vector.reciprocal_approx_fast(rs[:], sqv[:])
                        rs_b = work.tile([128, 512], F32, tag="rs_b")
                        nc.gpsimd.partition_broadcast(rs_b[:], rs[:])
                        t2 = work.tile([128, 512], F32, tag="t2")
                        nc.vector.tensor_mul(t2[:], ssw[:], sin_sb[:, ts(g, 512)])
                        tmp = work.tile([128, 512], F32, tag="tmp")
                        nc.vector.tensor_add(tmp[:], t1[:], t2[:])
                        dst = (QT[:, cb, ts(g, 512)] if cb < HQC
                               else KTb[:, cb - HQC, ts(g, 512)])
                        nc.vector.tensor_mul(dst, tmp[:], rs_b[:])
                    return tail

                for g in range(NG):
                    s, e = se[g]
                    wA, wB = e, 512 - s
                    xg_sb = xpool.tile([128, KT, 512], BF16, tag="xg")
                    if g == 0:
                        nc.sync.dma_start(xg_sb[:, 0:4], xg.ap()[g, :, 0:4])
                        nc.sync.dma_start(xg_sb[:, 4:KT], xg.ap()[g, :, 4:KT])
                        nc.sync.dma_start(cos_sb[:], cosT[:, :])
                        nc.sync.dma_start(sin_sb[:], sinT[:, :])
                        for cb in range(5, NCB):
                            nc.sync.dma_start(w0_sb[:, cb], w0.ap()[:, cb])
                            nc.sync.dma_start(w1_sb[:, cb], w1.ap()[:, cb])
                    else:
                        nc.sync.dma_start(xg_sb[:], xg.ap()[g])

                    for cb in range(NCB):
                        psA = psB = None
                        if wA:
                            psA = qkps.tile([128, 512], F32, tag="psA")
                        if wB:
                            psB = qkps.tile([128, 512], F32, tag="psB")
                        for k in range(KT):
                            st, sp = k == 0, k == KT - 1
                            if wA:
                                nc.tensor.matmul(
                                    psA[:, 0:wA], w0_sb[:, cb, k, :],
                                    xg_sb[:, k, 0:wA], start=st, stop=sp)
                            if wB:
                                nc.tensor.matmul(
                                    psB[:, 0:wB], w1_sb[:, cb, k, :],
                                    xg_sb[:, k, s:512], start=st, stop=sp)
                        # deferred PE + chains of the previous unit run here,
                        # covered by this unit's matmuls
                        flush()

                        # evict with expert select on the [s, e) window
                        sel = selp.tile([128, 512], F32, tag="sel")
                        if s > 0:
                            nc.vector.tensor_copy(sel[:, 0:s], psA[:, 0:s])
                        if e < 512:
                            nc.vector.tensor_copy(sel[:, e:512], psB[:, e - s:512 - s])
                        if e > s:
                            # DVE may read only one PSUM operand per op
                            wtB = work.tile([128, WSEL], F32, tag="wtB")
                            nc.vector.tensor_copy(wtB[:, 0:e - s], psB[:, 0:e - s])
                            wt = work.tile([128, WSEL], F32, tag="wt")
                            nc.vector.tensor_sub(
                                wt[:, 0:e - s], wtB[:, 0:e - s], psA[:, s:e])
                            nc.vector.tensor_mul(
                                wt[:, 0:e - s], wt[:, 0:e - s],
                                mpcw_sb[:, g, 0:e - s])
                            nc.vector.tensor_add(
                                sel[:, s:e], psA[:, s:e], wt[:, 0:e - s])

                        if cb < HQC + HKC:
                            # q/k: rms stats + rope now; finish deferred
                            sq = selp.tile([128, 512], BF16, tag="sq")
                            nc.scalar.activation(
                                sq[:], sel[:],
                                mybir.ActivationFunctionType.Square)
                            if has_qkw:
                                # norm weight applies before rope (rope mixes
                                # hd pairs); rms stats are pre-weight
                                nc.vector.tensor_mul(
                                    sel[:], sel[:],
                                    qkw_sb[:, 0 if cb < HQC else 1, ts(g, 512)])
                            ssw = selp.tile([128, 512], F32, tag="ssw")
                            nc.sync.dma_start(ssw[0:64, :], sel[64:128, :])
                            nc.sync.dma_start(ssw[64:128, :], sel[0:64, :])
                            t1 = work.tile([128, 512], F32, tag="t1")
                            nc.vector.tensor_mul(t1[:], sel[:], cos_sb[:, ts(g, 512)])

                            cs_ps = csps.tile([1, 512], F32, tag="cs")

                            def colsum(cs_ps=cs_ps, sq=sq):
                                nc.tensor.matmul(cs_ps[:], ones_col[:], sq[:],
                                                 start=True, stop=True)
                            pend2.append(colsum)
                            pend3.append(make_qk_tail(g, cb, t1, ssw, cs_ps))
                        else:
                            # v: transpose back to natural layout
                            vcb = cb - HQC - HKC
                            tp = tps.tile([128, 4, 128], F32, tag="tp")

                            def vtrans(tp=tp, sel=sel):
                                for j in range(4):
                                    nc.tensor.transpose(
                                        tp[:, j, :], sel[:, ts(j, 128)], ident[:])
                            pend2.append(vtrans)

                            def vcopy(tp=tp, g=g, vcb=vcb):
                                nc.vector.tensor_copy(
                                    Vb[:, ds(4 * g, 4), ts(vcb, 128)], tp[:])
                            pend3.append(vcopy)
                flush()

            # ------------- Phase 2+3: attention + wo + RS + final norm -------
            with ExitStack() as p23:
                wopool = p23.enter_context(tc.tile_pool(name="wopool", bufs=1))
                ofp = p23.enter_context(tc.tile_pool(name="ofp", bufs=1))
                sps = p23.enter_context(tc.tile_pool(name="sps", bufs=2, space="PSUM"))
                otps = p23.enter_context(tc.tile_pool(name="otps", bufs=1, space="PSUM"))
                dnps = p23.enter_context(tc.tile_pool(name="dnps", bufs=1, space="PSUM"))
                wops = p23.enter_context(tc.tile_pool(name="wops", bufs=2, space="PSUM"))
                probs = p23.enter_context(tc.tile_pool(name="probs", bufs=8))
                redc = p23.enter_context(tc.tile_pool(name="redc", bufs=3))
                att = p23.enter_context(tc.tile_pool(name="att", bufs=2))
                opool = p23.enter_context(tc.tile_pool(name="opool", bufs=3))
                npool = p23.enter_context(tc.tile_pool(name="npool", bufs=2))

                ofT = ofp.tile([128, HQC, SEQ], BF16)   # out_flat^T (hd, tok)
                nc.gpsimd.dma_start(dmasks[:], dmin.ap().rearrange("t p f -> p t f"))

                wo0_sb = wopool.tile([128, 4, DIM], BF16)
                nc.sync.dma_start(wo0_sb[:], wo0.ap().rearrange("(k p) f -> p k f", p=128))
                wo1_sb = wopool.tile([128, 4, DIM], BF16)
                nc.sync.dma_start(wo1_sb[:], wo1.ap().rearrange("(k p) f -> p k f", p=128))
                if has_anw:
                    anw0_sb = wopool.tile([1, DIM], F32)
                    nc.sync.dma_start(anw0_sb[:], anw0[:, :])
                    anwd_sb = wopool.tile([1, DIM], F32)
                    nc.sync.dma_start(anwd_sb[:], anwd[:, :])
                    anw0_b = wopool.tile([128, DIM], F32)
                    nc.gpsimd.partition_broadcast(anw0_b[:], anw0_sb[:])
                    anwd_b = wopool.tile([128, DIM], F32)
                    nc.gpsimd.partition_broadcast(anwd_b[:], anwd_sb[:])
                    mfin_sb = wopool.tile([128, NG + 1], F32)
                    nc.sync.dma_start(mfin_sb[:], mfin[:, :])

                pending_rs = []
                pending_den = []

                def do_den(g, h, dn_ps):
                    den = att.tile([1, 512], F32, tag="den")
                    nc.vector.reciprocal_approx_fast(den[:], dn_ps[:])
                    den_b = att.tile([128, 512], F32, tag="den_b")
                    nc.gpsimd.partition_broadcast(den_b[:], den[:])
                    nc.vector.tensor_mul(
                        ofT[:, h, ts(g, 512)], ofT[:, h, ts(g, 512)], den_b[:])

                def do_final_norm(row0, nrows, mf, rs_out, dep=None):
                    sum_sb = npool.tile([nrows, DIM], BF16, tag=f"sum_sb{nrows}")
                    first = nc.sync.dma_start(sum_sb[:], rs_out[:])
                    if dep is not None:
                        _add_dep_helper(first.ins, dep.ins, sync=False,
                                        reason="norms run only after all wo chunks")
                    fin = npool.tile([nrows, DIM], F32, tag=f"fin{nrows}")
                    z = npool.tile([nrows, 1], F32, tag=f"z{nrows}")
                    nc.vector.scalar_tensor_tensor(
                        out=fin[:], in0=sum_sb[:], scalar=1.0, in1=sum_sb[:],
                        op0=MUL, op1=MUL, accum_out=z[:])
                    sz = npool.tile([nrows, 1], F32, tag=f"sz{nrows}")
                    nc.scalar.activation(sz[:], z[:],
                                         mybir.ActivationFunctionType.Sqrt,
                                         scale=1.0 / float(DIM),
                                         bias=eps_1[0:nrows, :])
                    rz = npool.tile([nrows, 1], F32, tag=f"rz{nrows}")
                    nc.vector.reciprocal_approx_fast(rz[:], sz[:])
                    nc.scalar.mul(fin[:], sum_sb[:], rz[:])
                    if has_anw:
                        anw_sel = npool.tile([nrows, DIM], F32, tag="anw_sel")
                        nc.vector.scalar_tensor_tensor(
                            out=anw_sel[:], in0=anwd_b[0:nrows, :],
                            scalar=mfin_sb[0:nrows, mf:mf + 1],
                            in1=anw0_b[0:nrows, :], op0=MUL, op1=ADD)
                        nc.vector.tensor_mul(fin[:], fin[:], anw_sel[:])
                    nc.sync.dma_start(out_dram.ap()[ds(row0, nrows), :], fin[:])

                for g in range(NG):
                    for h in range(HQC):
                        kv = h // (HQC // HKC)
                        njt = 4 * (g + 1)
                        ot_ps = otps.tile([128, 512], F32, tag="ot")
                        dn_ps = dnps.tile([1, 512], F32, tag="dn")
                        dn_hold = []
                        dn_first = True
                        for jp in range(njt // 2):
                            j0 = 2 * jp
                            # two 512-score tiles in one 2-bank psum tile so
                            # exp and the causal-mask multiply run 1024 wide
                            s_ps = sps.tile([128, 2, 512], F32, tag="s")
                            for dj in range(2):
                                nc.tensor.matmul(
                                    s_ps[:, dj, :], KTb[:, kv, ts(j0 + dj, 128)],
                                    QT[:, h, ts(g, 512)], start=True, stop=True)
                            p_t = probs.tile([128, 2, 512], BF16, tag="p")
                            nc.scalar.activation(
                                p_t[:], s_ps[:], mybir.ActivationFunctionType.Exp)
                            if j0 >= 4 * g:
                                pm_t = probs.tile([128, 2, 512], BF16, tag="pm")
                                nc.vector.tensor_mul(
                                    pm_t[:], p_t[:],
                                    dmasks[:, ds(j0, 2), :])
                                p_t = pm_t
                            # denominator: DVE-pre-reduce up to 8 key tiles,
                            # then one 512-row matmul per batch
                            ps_pair = redc.tile([128, 512], BF16, tag="pp",
                                                bufs=4)
                            nc.vector.tensor_add(
                                ps_pair[:], p_t[:, 0, :], p_t[:, 1, :])
                            dn_hold.append(ps_pair)
                            if len(dn_hold) == 4 or jp == njt // 2 - 1:
                                while len(dn_hold) > 1:
                                    a = dn_hold.pop(0)
                                    b = dn_hold.pop(0)
                                    ps4 = redc.tile([128, 512], BF16, tag="p4")
                                    nc.vector.tensor_add(ps4[:], a[:], b[:])
                                    dn_hold.append(ps4)
                                nc.tensor.matmul(
                                    dn_ps[:], ones_col[:], dn_hold.pop()[:],
                                    start=dn_first,
                                    stop=(jp == njt // 2 - 1))
                                dn_first = False
                            for dj in range(2):
                                j = j0 + dj
                                st = j == 0
                                sp = j == njt - 1
                                nc.tensor.matmul(
                                    ot_ps[:], Vb[:, j, ts(kv, 128)], p_t[:, dj, :],
                                    start=st, stop=sp)
                        # fast raw evict frees the psum; normalization deferred
                        nc.vector.tensor_copy(ofT[:, h, ts(g, 512)], ot_ps[:])
                        pending_den.append((h, dn_ps))
                        if len(pending_den) > 1:
                            do_den(g, *pending_den.pop(0))

                    while pending_den:
                        do_den(g, *pending_den.pop(0))

                    # wo projection for this chunk (two small tail RS proved
                    # slower than one 2MB RS: ~10us fixed cost per collective)
                    nch = 1
                    for ch in range(nch):
                        nt_ch = 4 // nch
                        rs_in = dram.tile([128 * nt_ch, DIM], BF16,
                                          tag=f"rs_in{nt_ch}", bufs=4)
                        for u in range(nt_ch):
                            T = 4 * g + nt_ch * ch + u
                            kind = kinds[T]
                            woa_sb = wo1_sb if kind == 1 else wo0_sb
                            o_sb = opool.tile([128, DIM], BF16, tag="o_sb")
                            for n in range(4):
                                wo_ps = wops.tile([128, 512], F32, tag="wop")
                                if kind == 2:
                                    wb_ps = wops.tile([128, 512], F32, tag="wop")
                                for kk in range(4):
                                    nc.tensor.matmul(
                                        wo_ps[:], ofT[:, kk, ts(T, 128)],
                                        woa_sb[:, kk, ts(n, 512)],
                                        start=(kk == 0), stop=(kk == 3))
                                    if kind == 2:
                                        nc.tensor.matmul(
                                            wb_ps[:], ofT[:, kk, ts(T, 128)],
                                            wo1_sb[:, kk, ts(n, 512)],
                                            start=(kk == 0), stop=(kk == 3))
                                if kind == 2:
                                    nc.scalar.mul(o_sb[:, ts(n, 512)], wo_ps[:],
                                                  mpc1_sb[:, T:T + 1])
                                    nc.vector.scalar_tensor_tensor(
                                        out=o_sb[:, ts(n, 512)], in0=wb_ps[:],
                                        scalar=mpc_sb[:, T:T + 1],
                                        in1=o_sb[:, ts(n, 512)], op0=MUL, op1=ADD)
                                else:
                                    nc.scalar.copy(o_sb[:, ts(n, 512)], wo_ps[:])
                            last_rsin_dma = nc.sync.dma_start(
                                rs_in[ts(u, 128), :], o_sb[:])

                        rs_out = dram.tile([32 * nt_ch, DIM], BF16,
                                           tag=f"rs_out{nt_ch}", bufs=4)
                        nc.gpsimd.collective_compute(
                            "ReduceScatter", mybir.AluOpType.add,
                            replica_groups=GROUPS,
                            ins=[rs_in.opt()], outs=[rs_out.opt()])
                        mf = g if g < NG - 1 else NG - 1 + ch
                        pending_rs.append(
                            (128 * g + 32 * nt_ch * ch, 32 * nt_ch, mf, rs_out))

                # ALL final norms run after the last wo: nothing consumes a
                # ReduceScatter result until every chunk is in flight, so the
                # in-order engine queues never block on a collective even when
                # peer cores launch with large skew (the first RS would
                # otherwise wait for laggards and stall attention mid-phase).
                # The Tile scheduler reorders by priority, so emission order
                # alone is not enough: push priority to the maximum (appear
                # last) and chain the first DMA after the final wo output.
                with tc.high_priority(offset=-(1 << 20)):
                    for args in pending_rs:
                        do_final_norm(*args, dep=last_rsin_dma)

    nc.compile()
    return nc


def _plan(modality_ids):
    """Per-group stable modality sort; shared split points across batches."""
    mids = np.asarray(modality_ids).reshape(BS, SEQ)
    perms = np.empty((BS, SEQ), np.int64)   # permuted pos -> original token idx
    c0 = np.empty((BS, NG), np.int64)
    for b in range(BS):
        for G in range(NG):
            mg = mids[b, 512 * G:512 * (G + 1)]
            i0 = np.where(mg == 0)[0]
            i1 = np.where(mg == 1)[0]
            c0[b, G] = len(i0)
            perms[b, 512 * G:512 * (G + 1)] = 512 * G + np.concatenate([i0, i1])
    se = tuple((int(c0[:, G].min()), int(c0[:, G].max())) for G in range(NG))
    return perms, se


def _prep_inputs(x, freqs_cos, freqs_sin, wq, wk, wv, wo,
                 q_norm_w, k_norm_w, attn_norm_w, modality_ids,
                 has_qkw, has_anw, perms, se):
    """Build the 8 per-core input maps (numpy marshaling only)."""
    x = np.asarray(x, np.float32)
    freqs_cos = np.asarray(freqs_cos, np.float32)
    freqs_sin = np.asarray(freqs_sin, np.float32)
    wq = np.asarray(wq, np.float32)
    wk = np.asarray(wk, np.float32)
    wv = np.asarray(wv, np.float32)
    wo = np.asarray(wo, np.float32)
    mids = np.asarray(modality_ids).reshape(BS, SEQ)

    WSEL = 128
    while any(e - s > WSEL for s, e in se):
        WSEL *= 2

    # de-interleave the hd dimension: [even dims, odd dims]
    perm_hd = np.concatenate([np.arange(0, HD, 2), np.arange(1, HD, 2)])

    def permute_heads(w, nh):
        w4 = w.reshape(E, DIM, nh, HD)
        return w4[:, :, :, perm_hd].reshape(E, DIM, nh * HD)

    wq_p = permute_heads(wq, HQ)
    wk_p = permute_heads(wk, HK)
    wv_p = permute_heads(wv, HK)
    wo4 = wo.reshape(E, HQ, HD, DIM)[:, :, perm_hd, :].reshape(E, HQ * HD, DIM)

    cosT_full = np.concatenate([freqs_cos.T, freqs_cos.T], axis=0)   # (HD, SEQ)
    sinT_full = np.concatenate([-freqs_sin.T, freqs_sin.T], axis=0)

    in_maps = []
    for c in range(N_CORES):
        b, r = divmod(c, TP)
        P = perms[b]
        qs = slice(r * DQ, (r + 1) * DQ)
        ks = slice(r * DKV, (r + 1) * DKV)
        w0c = np.concatenate(
            [wq_p[0][:, qs], wk_p[0][:, ks], wv_p[0][:, ks]], axis=1)
        w1c = np.concatenate(
            [wq_p[1][:, qs], wk_p[1][:, ks], wv_p[1][:, ks]], axis=1)
        m = mids[b].astype(np.float32)[P]
        # in-group causal masks for the permuted order
        pos = (P % 512)
        dmv = np.zeros((NT, 128, 512), np.float32)
        for j in range(NT):
            gj = j // 4
            kpos = pos[128 * j:128 * (j + 1)]
            qpos = pos[512 * gj:512 * (gj + 1)]
            dmv[j] = (kpos[:, None] <= qpos[None, :])
        # expert-select window masks (broadcast down partitions)
        mw = np.zeros((128, NG, WSEL), np.float32)
        for G in range(NG):
            s, e = se[G]
            mw[:, G, 0:e - s] = m[512 * G + s:512 * G + e][None, :]
        xp = x[b].T[:, P]   # (DIM, SEQ) permuted tokens
        im = {
            "xg": np.ascontiguousarray(
                xp.reshape(KT, 128, NG, 512).transpose(2, 1, 0, 3)
            ).astype(ml_dtypes.bfloat16),
            "w0": np.ascontiguousarray(
                w0c.reshape(KT, 128, NCB, 128).transpose(1, 2, 0, 3)
            ).astype(ml_dtypes.bfloat16),
            "w1": np.ascontiguousarray(
                w1c.reshape(KT, 128, NCB, 128).transpose(1, 2, 0, 3)
            ).astype(ml_dtypes.bfloat16),
            "wo0": wo4[0][r * DQ:(r + 1) * DQ, :].astype(ml_dtypes.bfloat16),
            "wo1": wo4[1][r * DQ:(r + 1) * DQ, :].astype(ml_dtypes.bfloat16),
            "cosT": np.ascontiguousarray(cosT_full[:, P]),
            "sinT": np.ascontiguousarray(sinT_full[:, P]),
            "mpcw": mw,
            "mpc": np.ascontiguousarray(m.reshape(NT, 128).T),
            "mpc1": np.ascontiguousarray((1.0 - m).reshape(NT, 128).T),
            "dmin": dmv.astype(ml_dtypes.bfloat16),
        }
        if has_qkw:
            qw = np.asarray(q_norm_w, np.float32)[:, perm_hd]
            kw = np.asarray(k_norm_w, np.float32)[:, perm_hd]
            msel = mids[b][P]
            qkwv = np.empty((128, 2, SEQ), np.float32)
            qkwv[:, 0, :] = qw[msel].T
            qkwv[:, 1, :] = kw[msel].T
            im["qkw"] = qkwv
        if has_anw:
            aw = np.asarray(attn_norm_w, np.float32)
            im["anw0"] = np.ascontiguousarray(aw[0:1])
            im["anwd"] = (aw[1] - aw[0]).reshape(1, DIM).copy()
            mf = np.zeros((128, NG + 1), np.float32)
            for g in range(NG):
                t0 = 512 * g + 128 * r
                mf[:, g] = m[t0:t0 + 128]
            im["mfin"] = mf
        in_maps.append(im)
    return in_maps


def kernel(**inputs):
    q_norm_w = np.asarray(inputs["q_norm_w"], np.float32)
    k_norm_w = np.asarray(inputs["k_norm_w"], np.float32)
    attn_norm_w = np.asarray(inputs["attn_norm_w"], np.float32)
    has_qkw = not (np.all(q_norm_w == 1.0) and np.all(k_norm_w == 1.0))
    has_anw = not np.all(attn_norm_w == 1.0)

    perms, se = _plan(inputs["modality_ids"])
    key = (has_qkw, has_anw, se)
    if key not in _BUILD_CACHE:
        _BUILD_CACHE[key] = build_nc(has_qkw, has_anw, se)
    nc = _BUILD_CACHE[key]

    in_maps = _prep_inputs(
        inputs["x"], inputs["freqs_cos"], inputs["freqs_sin"],
        inputs["wq"], inputs["wk"], inputs["wv"], inputs["wo"],
        q_norm_w, k_norm_w, attn_norm_w, inputs["modality_ids"],
        has_qkw, has_anw, perms, se)

    res = run_bass_kernel_spmd(nc, in_maps, core_ids=list(range(N_CORES)))

    out = np.empty((BS, SEQ, DIM), np.float32)
    for c in range(N_CORES):
        b, r = divmod(c, TP)
        P = perms[b]
        oc = res.results[c]["out"]          # (SEQ//4, DIM), permuted rows
        for g in range(NG):
            t0 = 512 * g + 128 * r          # permuted-space positions
            out[b, P[t0:t0 + 128], :] = oc[128 * g:128 * (g + 1), :]
    return out
